# revision 1
# baseline (speedup 1.0000x reference)
"""GAT (2-layer dense-graph attention over 4096 nodes) as a Trainium2
Bass/Tile SPMD kernel across 8 NeuronCores.

Sharding: attention destination rows are sharded 512/core for both layers.
Each core computes the full source-side quantities (h', d — tiny) from the
full x, and the s-scores only for its own 512 destination rows. The layer-0
output (transposed) is exchanged between layers with FOUR chunked
AllGathers (2 heads = 16 feature rows each) so gather latency overlaps the
remaining heads' attention compute.

Math notes (exactness): softmax_j(leakyrelu(s_i+d_j)) is invariant to any
per-row factor, so with E = exp(leakyrelu(z)) = max(e^z, e^{0.2 z}) we use
E' = E * e^{-0.2 s_i} = max(e^{0.8 s_i} e^{d_j}, e^{0.2 d_j}),
computed as ONE fused DVE tensor_scalar op per [128, 512] tile:
(a_tile * b_j) max c_j, with a = e^{0.8 s} replicated across partitions and
b = e^d, c = e^{0.2 d} as per-partition scalars. BatchNorm (eval mode) is
folded into the weights host-side.

Precision/perf: E is bf16 (single-pass PE matmuls instead of the fp32
LOW_HIGH double-pass; bf16 quantization of E largely cancels between the
softmax numerator and denominator). The aggregation values h' are kept at
~fp32 precision by splitting into bf16 high + bf16 residual parts placed at
partition-aligned stationary columns (0/32) with the softmax-denominator
ones-column at 64 — matmul cost is N-bound, so the extra columns are free.
Compute engines can only address partition bases 0/32/64/96, which dictates
those offsets; partition-shifted row assembly goes through sbuf->sbuf DMA.
"""

import numpy as np
import ml_dtypes

import concourse.bacc as bacc
import concourse.mybir as mybir
import concourse.tile as tile
from concourse import masks
from concourse.bass_utils import run_bass_kernel_spmd

F32 = mybir.dt.float32
BF16 = mybir.dt.bfloat16
N = 4096
NCORES = 8
RPC = N // NCORES          # destination rows per core = 512
NJT = N // 128             # 32 j-tiles of 128 source rows
BN_EPS = 1e-5

_CACHE = {}


def _build():
    nc = bacc.Bacc("TRN2", target_bir_lowering=False, debug=False,
                   num_devices=NCORES)

    x_d = nc.dram_tensor("x", [N, 32], F32, kind="ExternalInput")
    xs_d = nc.dram_tensor("x_slice", [RPC, 32], F32, kind="ExternalInput")
    w0all_d = nc.dram_tensor("w0all", [33, 80], F32, kind="ExternalInput")
    w0s_d = nc.dram_tensor("w0s", [33, 8], F32, kind="ExternalInput")
    w1all_d = nc.dram_tensor("w1all", [65, 33], F32, kind="ExternalInput")
    w1b_d = nc.dram_tensor("w1b", [1, 33], F32, kind="ExternalInput")
    w1sc_d = nc.dram_tensor("w1sc", [16, 4], F32, kind="ExternalInput")
    sb1_d = nc.dram_tensor("sb1t", [1, 1], F32, kind="ExternalInput")
    b0cc_d = nc.dram_tensor("b0cc", [16, 4], F32, kind="ExternalInput")
    b1_d = nc.dram_tensor("b1f", [32, 1], F32, kind="ExternalInput")
    sela_d = nc.dram_tensor("sela", [8, 8 * 128], BF16, kind="ExternalInput")
    s2sel_d = nc.dram_tensor("s2sel", [2, 16], F32, kind="ExternalInput")
    out_d = nc.dram_tensor("out", [RPC, 32], F32, kind="ExternalOutput")

    with tile.TileContext(nc) as tc:
        with (
            tc.tile_pool(name="const", bufs=1) as const,
            tc.tile_pool(name="persist", bufs=1) as per,
            tc.tile_pool(name="dram", bufs=1, space="DRAM") as dram,
        ):
            ident = const.tile([128, 128], F32)
            masks.make_identity(nc, ident[:])
            ones_row = const.tile([1, 128], F32)
            nc.vector.memset(ones_row[:], 1.0)
            ones_row_bf = const.tile([1, 128], BF16)
            nc.vector.memset(ones_row_bf[:], 1.0)
            ones512 = const.tile([1, 512], F32)
            nc.vector.memset(ones512[:], 1.0)
            sela = const.tile([8, 8 * 128], BF16)
            nc.sync.dma_start(sela[:], sela_d[:])
            s2sel = const.tile([2, 16], F32)
            nc.sync.dma_start(s2sel[:], s2sel_d[:])

            w0all = const.tile([33, 80], F32)
            nc.sync.dma_start(w0all[:], w0all_d[:])
            w0s = const.tile([33, 8], F32)
            nc.sync.dma_start(w0s[:], w0s_d[:])
            w1all = const.tile([65, 33], F32)
            nc.sync.dma_start(w1all[:], w1all_d[:])
            w1b = const.tile([1, 33], F32)
            nc.sync.dma_start(w1b[:], w1b_d[:])
            w1sc = const.tile([16, 4], F32)
            nc.sync.dma_start(w1sc[:], w1sc_d[:])
            sb1t = const.tile([1, 1], F32)
            nc.sync.dma_start(sb1t[:], sb1_d[:])
            b0cc = const.tile([16, 4], F32)
            nc.sync.dma_start(b0cc[:], b0cc_d[:])
            b1c = const.tile([32, 1], F32)
            nc.sync.dma_start(b1c[:], b1_d[:])

            # big persistent sbuf tensors
            xT = per.tile([33, N], F32)        # x^T plus ones row
            xsT = per.tile([33, RPC], F32)     # x_slice^T plus ones row
            # stationary operand per (jt, h): hi(0:8) res(32:40) ones(64)
            hpa0 = per.tile([128, NJT, 8, 66], BF16)
            d0e = per.tile([128, NJT, 8], F32)       # e^{d0}
            d0e2 = per.tile([128, NJT, 8], F32)      # e^{0.2 d0}
            atile = per.tile([128, 8, 512], BF16)    # e^{0.8 s0} bcast
            outTNc = per.tile([16, 4, 512], F32)     # L0 numerators^T/chunk
            rowsc = per.tile([2, 4, 512], F32)       # L0 denominators/chunk
            contc = per.tile([16, 4, 512], F32)      # elu(out0)^T per chunk
            hTag = per.tile([65, 8, 512], F32)       # gathered h^T blocks
            # stationary per jt: hi(0:32) res(32:64) ones(64)
            hpa1 = per.tile([128, NJT, 66], BF16)
            d1e = per.tile([128, NJT], F32)
            d1e2 = per.tile([128, NJT], F32)
            a1tile = per.tile([128, 512], BF16)
            a0row = per.tile([8, 512], BF16)
            a1row = per.tile([1, 512], BF16)
            r1row = per.tile([1, 512], F32)
            num1 = per.tile([32, 512], F32)
            res1s = per.tile([32, 512], F32)
            norm1 = per.tile([32, 512], F32)

            contd = [dram.tile([16, 512], F32, name=f"contd{c}",
                               tag=f"contd{c}") for c in range(4)]
            agc = [dram.tile([NCORES * 16, 512], F32, name=f"agc{c}",
                             tag=f"agc{c}") for c in range(4)]

            # ---------------- Phase A: projections -----------------
            with (
                tc.tile_pool(name="ld", bufs=2) as ld,
                tc.tile_pool(name="tp", bufs=2, space="PSUM") as tp,
                tc.tile_pool(name="mm80", bufs=2, space="PSUM") as mm80,
                tc.tile_pool(name="pssa0", bufs=1, space="PSUM") as pssa0,
                tc.tile_pool(name="pssa", bufs=2, space="PSUM") as pssa,
                tc.tile_pool(name="wp", bufs=1, space="PSUM") as wp,
            ):
                # PE warm-up burst: ~20 back-to-back matmuls flip the HAM
                # clock gate to 8/8 while input DMAs are still in flight
                wsrc = ld.tile([128, 512], BF16, tag="wsrc")
                nc.vector.memset(wsrc[:], 0.5)
                wlhs = ld.tile([128, 128], BF16, tag="wlhs")
                nc.vector.memset(wlhs[:], 0.25)
                wps = wp.tile([128, 512], F32)
                for r in range(20):
                    nc.tensor.matmul(wps[:], wlhs[:], wsrc[:],
                                     start=(r == 0), stop=(r == 19))
                # x -> xT (32 transposes), x_slice -> xsT (4 transposes)
                xbig = ld.tile([128, NJT, 32], F32, tag="xbig")
                nc.sync.dma_start(
                    xbig[:], x_d[:].rearrange("(k p) c -> p k c", p=128))
                for k in range(NJT):
                    pt = tp.tile([32, 128], F32)
                    nc.tensor.matmul(pt[:], xbig[:, k, :], ident[:, :],
                                     is_transpose=True)
                    nc.vector.tensor_copy(xT[0:32, k * 128:(k + 1) * 128],
                                          pt[:])
                nc.vector.memset(xT[32:33, :], 1.0)

                xsbig = ld.tile([128, 4, 32], F32, tag="xsbig")
                nc.sync.dma_start(
                    xsbig[:], xs_d[:].rearrange("(k p) c -> p k c", p=128))
                for k in range(4):
                    pt = tp.tile([32, 128], F32)
                    nc.tensor.matmul(pt[:], xsbig[:, k, :], ident[:, :],
                                     is_transpose=True)
                    nc.vector.tensor_copy(xsT[0:32, k * 128:(k + 1) * 128],
                                          pt[:])
                nc.vector.memset(xsT[32:33, :], 1.0)

                # s0 rows for this core's 512 dst rows; a = e^{0.8 s}
                ps0 = pssa0.tile([8, 512], F32, tag="ps0")
                nc.tensor.matmul(ps0[:], w0s[:], xsT[:])
                nc.scalar.activation(a0row[:], ps0[:],
                                     mybir.ActivationFunctionType.Exp,
                                     scale=0.8)
                for h in range(8):
                    pa = pssa.tile([128, 512], F32, tag="pa")
                    nc.tensor.matmul(pa[:], sela[:, h * 128:(h + 1) * 128],
                                     a0row[:])
                    nc.vector.tensor_copy(atile[:, h, :], pa[:])

                # h'0 (hi+res), d0 exps per j-tile
                nc.vector.memset(hpa0[:], 0.0)
                nc.vector.memset(hpa0[:, :, :, 64:65], 1.0)
                for jt in range(NJT):
                    p80 = mm80.tile([128, 80], F32)
                    nc.tensor.matmul(p80[:], xT[:, jt * 128:(jt + 1) * 128],
                                     w0all[:])
                    hsrc = p80[:, 0:64].rearrange("p (h o) -> p h o", h=8)
                    nc.vector.tensor_copy(hpa0[:, jt, :, 0:8], hsrc)
                    # residual = fp32 h' - bf16(h')
                    nc.vector.tensor_tensor(hpa0[:, jt, :, 32:40], hsrc,
                                            hpa0[:, jt, :, 0:8],
                                            op=mybir.AluOpType.subtract)
                    nc.scalar.activation(d0e[:, jt, :], p80[:, 64:72],
                                         mybir.ActivationFunctionType.Exp)
                    nc.scalar.activation(d0e2[:, jt, :], p80[:, 64:72],
                                         mybir.ActivationFunctionType.Exp,
                                         scale=0.2)

            # ------- Phase B/C: layer-0 attention, chunked gather -------
            with (
                tc.tile_pool(name="epool", bufs=10) as epool,
                tc.tile_pool(name="agg", bufs=3, space="PSUM") as agg,
                tc.tile_pool(name="rb", bufs=2, space="PSUM") as rb,
                tc.tile_pool(name="tmp", bufs=2) as tmp,
            ):
                for h in range(8):
                    ch, hh = h // 2, h % 2
                    pg = agg.tile([65, 512], F32)
                    for jt in range(NJT):
                        e = epool.tile([128, 512], BF16, tag="e")
                        nc.vector.tensor_scalar(
                            e[:], atile[:, h, :],
                            d0e[:, jt, h:h + 1], d0e2[:, jt, h:h + 1],
                            op0=mybir.AluOpType.mult,
                            op1=mybir.AluOpType.max)
                        nc.tensor.matmul(pg[:], hpa0[:, jt, h, 0:65], e[:],
                                         start=(jt == 0), stop=(jt == NJT - 1))
                    # hi + residual numerators; engines address base 0/32/64
                    stgr = tmp.tile([8, 512], F32, tag="stgr")
                    nc.vector.tensor_copy(stgr[:], pg[32:40, :])
                    stgn = tmp.tile([8, 512], F32, tag="stgn")
                    nc.vector.tensor_tensor(stgn[:], pg[0:8, :], stgr[:],
                                            op=mybir.AluOpType.add)
                    stgd = tmp.tile([1, 512], F32, tag="stgd")
                    nc.vector.tensor_copy(stgd[:], pg[64:65, :])
                    nc.sync.dma_start(outTNc[hh * 8:(hh + 1) * 8, ch, :],
                                      stgn[:])
                    nc.sync.dma_start(rowsc[hh:hh + 1, ch, :], stgd[:])

                    if hh == 1:
                        # chunk ch complete: normalize + bias + ELU + gather
                        rrc = tmp.tile([2, 512], F32, tag="rrc")
                        nc.vector.reciprocal(rrc[:], rowsc[:, ch, :])
                        prb = rb.tile([16, 512], F32)
                        nc.tensor.matmul(prb[:], s2sel[:], rrc[:])
                        nrm = tmp.tile([16, 512], F32, tag="nrm")
                        nc.vector.tensor_tensor(nrm[:], outTNc[:, ch, :],
                                                prb[:],
                                                op=mybir.AluOpType.mult)
                        nc.vector.tensor_scalar_add(nrm[:], nrm[:],
                                                    b0cc[:, ch:ch + 1])
                        mneg = tmp.tile([16, 512], F32, tag="mneg")
                        nc.vector.tensor_scalar_min(mneg[:], nrm[:], 0.0)
                        eneg = tmp.tile([16, 512], F32, tag="eneg")
                        nc.scalar.activation(
                            eneg[:], mneg[:],
                            mybir.ActivationFunctionType.Exp)
                        ppos = tmp.tile([16, 512], F32, tag="ppos")
                        nc.vector.tensor_scalar_max(ppos[:], nrm[:], 0.0)
                        # elu = (eneg - 1) + ppos
                        nc.vector.scalar_tensor_tensor(
                            contc[:, ch, :], eneg[:], -1.0, ppos[:],
                            op0=mybir.AluOpType.add,
                            op1=mybir.AluOpType.add)
                        nc.sync.dma_start(contd[ch][:], contc[:, ch, :])
                        nc.gpsimd.collective_compute(
                            "AllGather",
                            mybir.AluOpType.bypass,
                            replica_groups=[list(range(NCORES))],
                            ins=[contd[ch].opt()],
                            outs=[agc[ch].opt()],
                        )
                        nc.sync.dma_start(
                            hTag[ch * 16:(ch + 1) * 16, :, :],
                            agc[ch][:].rearrange("(b r) f -> r b f", r=16))

                nc.vector.memset(hTag[64:65, :, :], 1.0)

            # ---------------- Phase D: layer 1 ----------------
            with (
                tc.tile_pool(name="e1pool", bufs=6) as e1pool,
                tc.tile_pool(name="mmd", bufs=2, space="PSUM") as mmd,
                tc.tile_pool(name="pd", bufs=1, space="PSUM") as pd,
                tc.tile_pool(name="agg1", bufs=1, space="PSUM") as agg1,
                tc.tile_pool(name="tp2", bufs=2, space="PSUM") as tp2,
                tc.tile_pool(name="ot", bufs=2) as ot,
            ):
                # s1 from the local contribution chunks (+ ones * sb1)
                ps1 = pd.tile([1, 512], F32, tag="ps1")
                for c in range(4):
                    nc.tensor.matmul(ps1[:], w1sc[:, c:c + 1],
                                     contc[:, c, :],
                                     start=(c == 0), stop=False)
                nc.tensor.matmul(ps1[:], sb1t[:], ones512[:],
                                 start=False, stop=True)
                nc.scalar.activation(a1row[:], ps1[:],
                                     mybir.ActivationFunctionType.Exp,
                                     scale=0.8)
                pa1 = pd.tile([128, 512], F32, tag="pa1")
                nc.tensor.matmul(pa1[:], ones_row_bf[:], a1row[:])
                nc.vector.tensor_copy(a1tile[:], pa1[:])

                nc.vector.memset(hpa1[:, :, 64:65], 1.0)
                for jt in range(NJT):
                    blk, kk = jt // 4, jt % 4
                    p34 = mmd.tile([128, 33], F32)
                    nc.tensor.matmul(
                        p34[:], hTag[:, blk, kk * 128:(kk + 1) * 128],
                        w1all[:])
                    nc.vector.tensor_copy(hpa1[:, jt, 0:32], p34[:, 0:32])
                    nc.vector.tensor_tensor(hpa1[:, jt, 32:64], p34[:, 0:32],
                                            hpa1[:, jt, 0:32],
                                            op=mybir.AluOpType.subtract)
                    nc.scalar.activation(d1e[:, jt:jt + 1], p34[:, 32:33],
                                         mybir.ActivationFunctionType.Exp)
                    nc.scalar.activation(d1e2[:, jt:jt + 1], p34[:, 32:33],
                                         mybir.ActivationFunctionType.Exp,
                                         scale=0.2)

                pg1 = agg1.tile([65, 512], F32)
                for jt in range(NJT):
                    e1 = e1pool.tile([128, 512], BF16, tag="e1")
                    nc.vector.tensor_scalar(
                        e1[:], a1tile[:],
                        d1e[:, jt:jt + 1], d1e2[:, jt:jt + 1],
                        op0=mybir.AluOpType.mult,
                        op1=mybir.AluOpType.max)
                    nc.tensor.matmul(pg1[:], hpa1[:, jt, 0:65], e1[:],
                                     start=(jt == 0), stop=(jt == NJT - 1))

                nc.vector.reciprocal(r1row[:], pg1[64:65, :])
                prb1 = pd.tile([32, 512], F32, tag="prb1")
                nc.tensor.matmul(prb1[:], ones_row[0:1, 0:32], r1row[:])
                nc.vector.tensor_copy(res1s[:], pg1[32:64, :])
                nc.vector.tensor_tensor(num1[:], pg1[0:32, :], res1s[:],
                                        op=mybir.AluOpType.add)
                nc.vector.tensor_tensor(norm1[:], num1[:], prb1[:],
                                        op=mybir.AluOpType.mult)
                nc.vector.tensor_scalar_add(norm1[:], norm1[:], b1c[:])

                for ic in range(4):
                    pt2 = tp2.tile([128, 32], F32)
                    nc.tensor.matmul(pt2[:],
                                     norm1[:, ic * 128:(ic + 1) * 128],
                                     ident[0:32, 0:32], is_transpose=True)
                    ob = ot.tile([128, 32], F32, tag="ob")
                    nc.vector.tensor_copy(ob[:], pt2[:])
                    nc.sync.dma_start(out_d[ic * 128:(ic + 1) * 128, :],
                                      ob[:])

    nc.compile()
    return nc


def _fold(inputs):
    """Host-side BN folding and attention-projection folding (numpy)."""
    f64 = np.float64
    x = np.ascontiguousarray(np.asarray(inputs["x"], np.float32))
    w0 = np.asarray(inputs["w0"], f64)          # [8, 32, 8]
    w1 = np.asarray(inputs["w1"], f64)          # [1, 64, 32]
    a_src0 = np.asarray(inputs["a_src0"], f64)[..., 0]   # [8, 8]
    a_dst0 = np.asarray(inputs["a_dst0"], f64)[..., 0]   # [8, 8]
    a_src1 = np.asarray(inputs["a_src1"], f64)[0, :, 0]  # [32]
    a_dst1 = np.asarray(inputs["a_dst1"], f64)[0, :, 0]  # [32]
    b0 = np.asarray(inputs["b0"], f64)          # [8]
    b1 = np.asarray(inputs["b1"], f64)          # [32]

    al0 = np.asarray(inputs["bn0_gamma"], f64) / np.sqrt(
        np.asarray(inputs["bn0_var"], f64) + BN_EPS)
    sh0 = np.asarray(inputs["bn0_beta"], f64) - \
        np.asarray(inputs["bn0_mean"], f64) * al0
    al1 = np.asarray(inputs["bn1_gamma"], f64) / np.sqrt(
        np.asarray(inputs["bn1_var"], f64) + BN_EPS)
    sh1 = np.asarray(inputs["bn1_beta"], f64) - \
        np.asarray(inputs["bn1_mean"], f64) * al1

    # layer 0 folds
    w0flat = (al0[None, :, None] * w0).transpose(1, 0, 2).reshape(32, 64)
    beta0h = np.einsum("i,hio->ho", sh0, w0)     # [8, 8]
    beta0 = beta0h.reshape(64)
    as0 = al0[:, None] * np.einsum("hio,ho->ih", w0, a_src0)   # [32, 8]
    sb0 = np.einsum("ho,ho->h", beta0h, a_src0)
    ad0 = al0[:, None] * np.einsum("hio,ho->ih", w0, a_dst0)
    db0 = np.einsum("ho,ho->h", beta0h, a_dst0)

    w0all = np.zeros((33, 80), f64)
    w0all[0:32, 0:64] = w0flat
    w0all[32, 0:64] = beta0
    w0all[0:32, 64:72] = ad0
    w0all[32, 64:72] = db0
    w0s = np.zeros((33, 8), f64)
    w0s[0:32, :] = as0
    w0s[32, :] = sb0

    # layer 1 folds
    w1m = w1[0]                                   # [64, 32]
    w1flat = al1[:, None] * w1m
    beta1 = sh1 @ w1m                             # [32]
    as1 = al1 * (w1m @ a_src1)
    sb1 = beta1 @ a_src1
    ad1 = al1 * (w1m @ a_dst1)
    db1 = beta1 @ a_dst1

    w1all = np.zeros((65, 33), f64)
    w1all[0:64, 0:32] = w1flat
    w1all[64, 0:32] = beta1
    w1all[0:64, 32] = ad1
    w1all[64, 32] = db1

    b0f = np.tile(b0, 8)                          # (h,o) flat -> b0[o]
    b0cc = b0f.reshape(4, 16).T                   # [16, 4] per chunk
    b1f = b1.reshape(32, 1)
    w1sc = as1.reshape(4, 16).T                   # [16, 4] per chunk
    sb1t = np.array([[sb1]])

    sela = np.zeros((8, 8, 128), ml_dtypes.bfloat16)  # row h ones in block h
    for h in range(8):
        sela[h, h, :] = 1.0
    s2sel = np.zeros((2, 16), np.float32)         # S[p, m] = (m//8 == p)
    for p in range(2):
        s2sel[p, p * 8:(p + 1) * 8] = 1.0

    return {
        "x": x,
        "w0all": w0all.astype(np.float32),
        "w0s": w0s.astype(np.float32),
        "w1all": w1all.astype(np.float32),
        "w1b": w1all[64:65, :].astype(np.float32),
        "w1sc": w1sc.astype(np.float32),
        "sb1t": sb1t.astype(np.float32),
        "b0cc": b0cc.astype(np.float32),
        "b1f": b1f.astype(np.float32),
        "sela": sela.reshape(8, 8 * 128),
        "s2sel": s2sel,
    }


def kernel(**inputs) -> np.ndarray:
    if "nc" not in _CACHE:
        _CACHE["nc"] = _build()
    nc = _CACHE["nc"]

    shared = _fold(inputs)
    x = shared["x"]
    in_maps = []
    for c in range(NCORES):
        m = dict(shared)
        m["x_slice"] = np.ascontiguousarray(x[c * RPC:(c + 1) * RPC])
        in_maps.append(m)

    res = run_bass_kernel_spmd(nc, in_maps, list(range(NCORES)))
    out = np.concatenate([res.results[c]["out"] for c in range(NCORES)],
                         axis=0)
    return out.astype(np.float32)



# revision 16
# speedup vs baseline: 1.2306x; 1.2306x over previous
"""GAT (2-layer dense-graph attention over 4096 nodes) as a Trainium2
Bass/Tile SPMD kernel across 8 NeuronCores.

Sharding: attention destination rows are sharded 512/core for both layers.
Each core computes the full source-side quantities (h', d) from the full x
and the s-scores only for its own 512 destination rows.

v2 design (from baseline trace analysis):
- No fp32-residual columns: pure-bf16 h' gives ~6e-4 rel err (gate 2e-2).
  Stationary per (jt, h) is 33 cols: h' at output partitions 0:7 (base 0),
  ones at 32 (base 32) -- engines can only address partition bases
  0/32/64/96, which pins those offsets.
- E' = max(e^{0.8 s_i} e^{d_j}, e^{0.2 d_j}) as one tensor_scalar per
  [128, 512] tile, split DVE (2/3) / GPSIMD (1/3) so tile production keeps
  pace with the PE and the HAM clock gate stays at 8/8 (2.4 GHz).
- x arrives host-pre-transposed (xT with bias row) -- no on-chip transposes.
- Per-chunk softmax-normalize + ELU is emitted interleaved into the NEXT
  head's tile stream so the in-order DVE queue never head-blocks the PE.
- Layer-1 projection h'1 = elu_h @ W1 runs LOCALLY on the owning core,
  PSUM-accumulated chunk-by-chunk during the L0 heads. After head 7 a
  single bf16 [512, 36] payload {h'1, ones, e^{d1}, e^{0.2 d1}} is
  AllGathered (vs 4 fp32 gathers + serial remote projection in v1).
- A dummy 1KB AllGather at kernel start absorbs the ~11 us
  first-collective setup penalty.
"""

import numpy as np
import ml_dtypes

import concourse.bacc as bacc
import concourse.mybir as mybir
import concourse.tile as tile
from concourse import masks
from concourse.bass_utils import run_bass_kernel_spmd

F32 = mybir.dt.float32
BF16 = mybir.dt.bfloat16
N = 4096
NCORES = 8
RPC = N // NCORES          # destination rows per core = 512
NJT = N // 128             # 32 j-tiles of 128 source rows
BN_EPS = 1e-5

_CACHE = {}
DEBUG = False


def _build():
    nc = bacc.Bacc("TRN2", target_bir_lowering=False, debug=False,
                   num_devices=NCORES)

    xT_d = nc.dram_tensor("xT33", [33, N], F32, kind="ExternalInput")
    xsT_d = nc.dram_tensor("xsT33", [33, RPC], F32, kind="ExternalInput")
    w0aug_d = nc.dram_tensor("w0aug", [33, 72], F32, kind="ExternalInput")
    w0s_d = nc.dram_tensor("w0s", [33, 8], F32, kind="ExternalInput")
    w1ch_d = nc.dram_tensor("w1ch", [16, 4 * 34], F32, kind="ExternalInput")
    w1b_d = nc.dram_tensor("w1b", [1, 34], F32, kind="ExternalInput")
    w1sc_d = nc.dram_tensor("w1sc", [16, 4], F32, kind="ExternalInput")
    sb1_d = nc.dram_tensor("sb1t", [1, 1], F32, kind="ExternalInput")
    b0p_d = nc.dram_tensor("b0p", [8, 1], F32, kind="ExternalInput")
    nb0p_d = nc.dram_tensor("nb0p", [8, 1], F32, kind="ExternalInput")
    b1_d = nc.dram_tensor("b1f", [32, 1], F32, kind="ExternalInput")
    sela_d = nc.dram_tensor("sela", [8, 8 * 128], BF16, kind="ExternalInput")
    out_d = nc.dram_tensor("out", [RPC, 32], F32, kind="ExternalOutput")
    if DEBUG:
        dbg_a0_d = nc.dram_tensor("dbg_a0", [8, 512], BF16,
                                  kind="ExternalOutput")
        dbg_ds_d = nc.dram_tensor("dbg_ds", [128, 8], F32,
                                  kind="ExternalOutput")
        dbg_hpa_d = nc.dram_tensor("dbg_hpa", [128, 34], BF16,
                                   kind="ExternalOutput")
        dbg_cont_d = nc.dram_tensor("dbg_cont", [16, 4 * 512], F32,
                                    kind="ExternalOutput")
        dbg_pay_d = nc.dram_tensor("dbg_pay", [36, 512], BF16,
                                   kind="ExternalOutput")
        dbg_hpa1_d = nc.dram_tensor("dbg_hpa1", [128, 36], BF16,
                                    kind="ExternalOutput")
        dbg_a1_d = nc.dram_tensor("dbg_a1", [1, 512], BF16,
                                  kind="ExternalOutput")
        dbg_pg1_d = nc.dram_tensor("dbg_pg1", [33, 512], F32,
                                   kind="ExternalOutput")

    with tile.TileContext(nc) as tc:
        with (
            tc.tile_pool(name="const", bufs=1) as const,
            tc.tile_pool(name="persist", bufs=1) as per,
            tc.tile_pool(name="pacc", bufs=1, space="PSUM") as pacc,
            tc.tile_pool(name="dram", bufs=1, space="DRAM") as dram,
        ):
            ident = const.tile([128, 128], F32)
            masks.make_identity(nc, ident[:])
            identB = const.tile([36, 36], BF16)
            nc.vector.tensor_copy(identB[:], ident[0:36, 0:36])
            ones8 = const.tile([1, 8], F32)
            nc.vector.memset(ones8[:], 1.0)
            ones32 = const.tile([1, 32], F32)
            nc.vector.memset(ones32[:], 1.0)
            ones512 = const.tile([1, 512], F32)
            nc.vector.memset(ones512[:], 1.0)
            ones_row_bf = const.tile([1, 128], BF16)
            nc.vector.memset(ones_row_bf[:], 1.0)
            sela = const.tile([8, 8 * 128], BF16)
            nc.sync.dma_start(sela[:], sela_d[:])

            w0aug = const.tile([33, 72], F32)
            nc.sync.dma_start(w0aug[:], w0aug_d[:])
            w0s = const.tile([33, 8], F32)
            nc.sync.dma_start(w0s[:], w0s_d[:])
            w1ch = const.tile([16, 4, 34], F32)
            nc.sync.dma_start(w1ch[:], w1ch_d[:].rearrange("p (c f) -> p c f", c=4))
            w1b = const.tile([1, 34], F32)
            nc.sync.dma_start(w1b[:], w1b_d[:])
            w1sc = const.tile([16, 4], F32)
            nc.sync.dma_start(w1sc[:], w1sc_d[:])
            sb1t = const.tile([1, 1], F32)
            nc.sync.dma_start(sb1t[:], sb1_d[:])
            b0p = const.tile([8, 1], F32)
            nc.sync.dma_start(b0p[:], b0p_d[:])
            nb0p = const.tile([8, 1], F32)
            nc.sync.dma_start(nb0p[:], nb0p_d[:])
            b1c = const.tile([32, 1], F32)
            nc.sync.dma_start(b1c[:], b1_d[:])

            # big persistent sbuf tensors
            xT = per.tile([33, N], F32)        # x^T plus ones row (from host)
            xsT = per.tile([33, RPC], F32)
            # stationary per (jt, h): h' bf16 at cols 0:8, ones col at 32
            hpa0 = per.tile([128, NJT, 8, 34], BF16)
            dstage = per.tile([128, NJT, 8], F32)    # raw d0 per (j, jt, h)
            d0e = per.tile([128, NJT, 8], F32)       # e^{d0}
            d0e2 = per.tile([128, NJT, 8], F32)      # e^{0.2 d0}
            atile = per.tile([128, 8, 512], BF16)    # e^{0.8 s0} bcast
            a0row = per.tile([8, 512], BF16)
            contc = per.tile([16, 4, 512], F32)      # elu(out0)^T per chunk
            # layer 1 stationary: h'1 bf16 0:32, ones 32, e^{d1} 33,
            # e^{0.2 d1} 34, pad 35
            hpa1 = per.tile([128, NJT, 36], BF16)
            d1e = per.tile([128, NJT, 1], F32)
            d1e2 = per.tile([128, NJT, 1], F32)
            a1tile = per.tile([128, 512], BF16)
            a1row = per.tile([1, 512], BF16)
            paySrc = per.tile([36, 512], BF16)
            payT = per.tile([128, 4, 36], BF16)
            ed1st = per.tile([1, 512], BF16)
            ed2st = per.tile([1, 512], BF16)
            o1s = per.tile([32, 512], F32)

            # PSUM accumulators alive across the whole heads phase
            p1T = pacc.tile([33, 512], F32, tag="p1T")   # local h'1^T (+d1)
            ps1 = pacc.tile([1, 512], F32, tag="ps1")    # local s1

            dumin = dram.tile([8, 16], F32, name="dumin", tag="dumin")
            dumout = dram.tile([NCORES * 8, 16], F32, name="dumout",
                               tag="dumout")
            contd1 = dram.tile([RPC, 36], BF16, name="contd1", tag="contd1")
            agc1 = dram.tile([N, 36], BF16, name="agc1", tag="agc1")

            # ---------------- Phase A: projections -----------------
            with (
                tc.tile_pool(name="ld", bufs=2) as ld,
                tc.tile_pool(name="mm80", bufs=2, space="PSUM") as mm80,
                tc.tile_pool(name="pssa0", bufs=1, space="PSUM") as pssa0,
                tc.tile_pool(name="pssa", bufs=2, space="PSUM") as pssa,
                tc.tile_pool(name="wp", bufs=1, space="PSUM") as wp,
            ):
                # dummy collective to absorb first-CC setup cost (overlaps A)
                dustage = ld.tile([8, 16], F32, tag="dustage")
                nc.vector.memset(dustage[:], 1.0)
                nc.sync.dma_start(dumin[:], dustage[:])
                nc.gpsimd.collective_compute(
                    "AllGather",
                    mybir.AluOpType.bypass,
                    replica_groups=[list(range(NCORES))],
                    ins=[dumin.opt()],
                    outs=[dumout.opt()],
                )

                # PE warm-up burst: back-to-back matmuls flip the HAM clock
                # gate to 8/8 while input DMAs are still in flight
                wsrc = ld.tile([128, 512], BF16, tag="wsrc")
                nc.vector.memset(wsrc[:], 0.5)
                wlhs = ld.tile([128, 128], BF16, tag="wlhs")
                nc.vector.memset(wlhs[:], 0.25)
                wps = wp.tile([128, 512], F32)
                for r in range(20):
                    nc.tensor.matmul(wps[:], wlhs[:], wsrc[:],
                                     start=(r == 0), stop=(r == 19))

                nc.sync.dma_start(xT[:], xT_d[:])
                nc.sync.dma_start(xsT[:], xsT_d[:])

                # zero-init big stationaries (ones cols set below)
                nc.vector.memset(hpa0[:], 0.0)
                nc.vector.memset(hpa0[:, :, :, 32:33], 1.0)
                nc.vector.memset(hpa1[:], 0.0)
                nc.vector.memset(paySrc[:], 0.0)

                # s0 rows for this core's 512 dst rows; a = e^{0.8 s}
                ps0 = pssa0.tile([8, 512], F32, tag="ps0")
                nc.tensor.matmul(ps0[:], w0s[:], xsT[:])
                nc.scalar.activation(a0row[:], ps0[:],
                                     mybir.ActivationFunctionType.Exp,
                                     scale=0.8)
                for h in range(8):
                    pa = pssa.tile([128, 512], F32, tag="pa")
                    nc.tensor.matmul(pa[:], sela[:, h * 128:(h + 1) * 128],
                                     a0row[:])
                    if h % 2 == 0:
                        nc.vector.tensor_copy(atile[:, h, :], pa[:])
                    else:
                        nc.scalar.copy(atile[:, h, :], pa[:])

                # h'0 (bf16) and raw d0 per j-tile
                for jt in range(NJT):
                    p80 = mm80.tile([128, 72], F32)
                    nc.tensor.matmul(p80[:], xT[:, jt * 128:(jt + 1) * 128],
                                     w0aug[:])
                    hsrc = p80[:, 0:64].rearrange("p (h o) -> p h o", h=8)
                    if jt % 2 == 0:
                        nc.vector.tensor_copy(hpa0[:, jt, :, 0:8], hsrc)
                        nc.scalar.copy(dstage[:, jt, :], p80[:, 64:72])
                    else:
                        nc.scalar.copy(hpa0[:, jt, :, 0:8], hsrc)
                        nc.vector.tensor_copy(dstage[:, jt, :], p80[:, 64:72])
                    if jt % 8 == 7:
                        b = jt - 7
                        nc.scalar.activation(
                            d0e[:, b:jt + 1, :], dstage[:, b:jt + 1, :],
                            mybir.ActivationFunctionType.Exp)
                        nc.scalar.activation(
                            d0e2[:, b:jt + 1, :], dstage[:, b:jt + 1, :],
                            mybir.ActivationFunctionType.Exp, scale=0.2)

            # ------- Phase B: layer-0 attention + local L1 projection -------
            with (
                tc.tile_pool(name="epool", bufs=14) as epool,
                tc.tile_pool(name="agg", bufs=3, space="PSUM") as agg,
                tc.tile_pool(name="prbp", bufs=2, space="PSUM") as prbp,
                tc.tile_pool(name="tmp", bufs=3) as tmp,
            ):
                pgs = {}

                def emit_head_tile(h, jt):
                    if jt == 0:
                        pgs[h] = agg.tile([33, 512], F32, name=f"pg{h}",
                                          tag="pg")
                    pg = pgs[h]
                    e = epool.tile([128, 512], BF16, tag="e")
                    nc.vector.tensor_scalar(
                        e[:], atile[:, h, :],
                        d0e[:, jt, h:h + 1], d0e2[:, jt, h:h + 1],
                        op0=mybir.AluOpType.mult,
                        op1=mybir.AluOpType.max)
                    nc.tensor.matmul(pg[:], hpa0[:, jt, h, 0:33], e[:],
                                     start=(jt == 0), stop=(jt == NJT - 1))

                def emit_norm(h, step):
                    """Normalize + bias + ELU for head h, split into 6 steps
                    so it interleaves with the next head's tile stream."""
                    ch, hh = h // 2, h % 2
                    pg = pgs[h]
                    st = norm_state[h]
                    if step == 0:
                        st['rcp'] = tmp.tile([1, 512], F32, name="rcp", tag="rcp")
                        nc.vector.reciprocal(st['rcp'][:], pg[32:33, :])
                    elif step == 1:
                        prb = prbp.tile([8, 512], F32)
                        nc.tensor.matmul(prb[:], ones8[:], st['rcp'][:])
                        st['prbs'] = tmp.tile([8, 512], F32, name="prbs", tag="prbs")
                        nc.scalar.copy(st['prbs'][:], prb[:])
                    elif step == 2:
                        st['nrm'] = tmp.tile([8, 512], F32, name="nrm", tag="nrm")
                        nc.vector.tensor_tensor(st['nrm'][:], pg[0:8, :],
                                                st['prbs'][:],
                                                op=mybir.AluOpType.mult)
                    elif step == 3:
                        # eneg = exp(min(nrm+b0, 0)) = exp(-relu(-nrm-b0))
                        st['mneg'] = tmp.tile([8, 512], F32, name="mneg", tag="mneg")
                        nc.scalar.activation(
                            st['mneg'][:], st['nrm'][:],
                            mybir.ActivationFunctionType.Relu,
                            bias=nb0p[:], scale=-1.0)
                        st['eneg'] = tmp.tile([8, 512], F32, name="eneg", tag="eneg")
                        nc.scalar.activation(
                            st['eneg'][:], st['mneg'][:],
                            mybir.ActivationFunctionType.Exp, scale=-1.0)
                    elif step == 4:
                        st['ppos'] = tmp.tile([8, 512], F32, name="ppos", tag="ppos")
                        nc.scalar.activation(
                            st['ppos'][:], st['nrm'][:],
                            mybir.ActivationFunctionType.Relu,
                            bias=b0p[:])
                    elif step == 5:
                        # elu half-row = (eneg - 1) + ppos
                        st['half'] = tmp.tile([8, 512], F32, name="half", tag="half")
                        nc.vector.scalar_tensor_tensor(
                            st['half'][:], st['eneg'][:], -1.0, st['ppos'][:],
                            op0=mybir.AluOpType.add,
                            op1=mybir.AluOpType.add)
                        nc.sync.dma_start(contc[hh * 8:(hh + 1) * 8, ch, :],
                                          st['half'][:])
                        if hh == 1:
                            # chunk complete: accumulate local L1 projection
                            # h'1^T += W1_ch^T @ contc_ch  and s1 partials
                            nc.tensor.matmul(p1T[:], w1ch[:, ch, 0:33],
                                             contc[:, ch, :],
                                             start=(ch == 0), stop=False)
                            nc.tensor.matmul(ps1[:], w1sc[:, ch:ch + 1],
                                             contc[:, ch, :],
                                             start=(ch == 0), stop=False)

                norm_state = [dict() for _ in range(8)]
                NORM_AT = [7, 11, 15, 19, 23, 27]  # jt positions in next head
                for h in range(8):
                    for jt in range(NJT):
                        emit_head_tile(h, jt)
                        if h > 0 and jt in NORM_AT:
                            emit_norm(h - 1, NORM_AT.index(jt))
                # head 7 norm: nothing left to interleave with
                for step in range(6):
                    emit_norm(7, step)

            # ---------------- Phase C: payload + gather ----------------
            with (
                tc.tile_pool(name="pd", bufs=2, space="PSUM") as pd,
                tc.tile_pool(name="tp2", bufs=2, space="PSUM") as tp2,
                tc.tile_pool(name="ot", bufs=2) as ot,
            ):
                # close the local L1 projection: bias row (beta1, db1) and
                # s1 bias, then exps
                nc.tensor.matmul(p1T[:], w1b[:, 0:33], ones512[:],
                                 start=False, stop=True)
                nc.tensor.matmul(ps1[:], sb1t[:], ones512[:],
                                 start=False, stop=True)
                nc.scalar.activation(a1row[:], ps1[:],
                                     mybir.ActivationFunctionType.Exp,
                                     scale=0.8)
                pa1 = pd.tile([128, 512], F32, tag="pa1")
                nc.tensor.matmul(pa1[:], ones_row_bf[:], a1row[:])
                nc.vector.tensor_copy(a1tile[:], pa1[:])

                # payload rows: 0:32 h'1 bf16, 32 ones, 33 e^{d1},
                # 34 e^{0.2 d1}, 35 pad
                nc.vector.tensor_copy(paySrc[0:32, :], p1T[0:32, :])
                nc.vector.memset(paySrc[32:33, :], 1.0)
                # engine writes are limited to partition bases 0/32/64/96:
                # stage the exps at base 0 and DMA them into rows 33/34
                nc.scalar.activation(ed1st[:], p1T[32:33, :],
                                     mybir.ActivationFunctionType.Exp)
                nc.scalar.activation(ed2st[:], p1T[32:33, :],
                                     mybir.ActivationFunctionType.Exp,
                                     scale=0.2)
                nc.sync.dma_start(paySrc[33:34, :], ed1st[:])
                nc.sync.dma_start(paySrc[34:35, :], ed2st[:])

                if DEBUG:
                    nc.sync.dma_start(dbg_a0_d[:], a0row[:])
                    nc.sync.dma_start(dbg_ds_d[:], dstage[:, 0, :])
                    nc.sync.dma_start(dbg_hpa_d[:], hpa0[:, 0, 0, :])
                    nc.sync.dma_start(
                        dbg_cont_d[:],
                        contc[:].rearrange("p c f -> p (c f)"))
                    nc.sync.dma_start(dbg_pay_d[:], paySrc[:])
                for k in range(4):
                    pt = tp2.tile([128, 36], BF16)
                    nc.tensor.matmul(pt[:],
                                     paySrc[:, k * 128:(k + 1) * 128],
                                     identB[:], is_transpose=True)
                    nc.vector.tensor_copy(payT[:, k, :], pt[:])
                nc.sync.dma_start(
                    contd1[:].rearrange("(k p) c -> p k c", p=128),
                    payT[:])
                nc.gpsimd.collective_compute(
                    "AllGather",
                    mybir.AluOpType.bypass,
                    replica_groups=[list(range(NCORES))],
                    ins=[contd1.opt()],
                    outs=[agc1.opt()],
                )
                nc.sync.dma_start(
                    hpa1[:],
                    agc1[:].rearrange("(k p) c -> p k c", p=128))
                nc.vector.tensor_copy(d1e[:], hpa1[:, :, 33:34])
                nc.vector.tensor_copy(d1e2[:], hpa1[:, :, 34:35])

            # ---------------- Phase D: layer-1 attention ----------------
            with (
                tc.tile_pool(name="e1pool", bufs=10) as e1pool,
                tc.tile_pool(name="agg1", bufs=1, space="PSUM") as agg1,
                tc.tile_pool(name="pd2", bufs=2, space="PSUM") as pd2,
                tc.tile_pool(name="tp3", bufs=2, space="PSUM") as tp3,
                tc.tile_pool(name="ot2", bufs=2) as ot2,
                tc.tile_pool(name="tmp2", bufs=2) as tmp2,
            ):
                pg1 = agg1.tile([33, 512], F32)
                for jt in range(NJT):
                    e1 = e1pool.tile([128, 512], BF16, tag="e1")
                    nc.vector.tensor_scalar(
                        e1[:], a1tile[:],
                        d1e[:, jt, :], d1e2[:, jt, :],
                        op0=mybir.AluOpType.mult,
                        op1=mybir.AluOpType.max)
                    nc.tensor.matmul(pg1[:], hpa1[:, jt, 0:33], e1[:],
                                     start=(jt == 0), stop=(jt == NJT - 1))

                if DEBUG:
                    nc.sync.dma_start(dbg_hpa1_d[:], hpa1[:, 0, :])
                    nc.sync.dma_start(dbg_a1_d[:], a1row[:])
                    pg1s = tmp2.tile([33, 512], F32, tag="pg1s")
                    nc.vector.tensor_copy(pg1s[:], pg1[:])
                    nc.sync.dma_start(dbg_pg1_d[:], pg1s[:])
                rcp1 = tmp2.tile([1, 512], F32, tag="rcp1")
                nc.vector.reciprocal(rcp1[:], pg1[32:33, :])
                prb1 = pd2.tile([32, 512], F32, tag="prb1")
                nc.tensor.matmul(prb1[:], ones32[:], rcp1[:])
                prbs1 = tmp2.tile([32, 512], F32, tag="prbs1")
                nc.scalar.copy(prbs1[:], prb1[:])
                nc.vector.tensor_tensor(o1s[:], pg1[0:32, :], prbs1[:],
                                        op=mybir.AluOpType.mult)
                nc.vector.tensor_scalar_add(o1s[:], o1s[:], b1c[:])

                for ic in range(4):
                    pt2 = tp3.tile([128, 32], F32)
                    nc.tensor.matmul(pt2[:],
                                     o1s[:, ic * 128:(ic + 1) * 128],
                                     ident[0:32, 0:32], is_transpose=True)
                    ob = ot2.tile([128, 32], F32, tag="ob")
                    nc.scalar.copy(ob[:], pt2[:])
                    nc.sync.dma_start(out_d[ic * 128:(ic + 1) * 128, :],
                                      ob[:])

    nc.compile()
    return nc


def _fold(inputs):
    """Host-side BN folding, attention-projection folding, x transpose."""
    f64 = np.float64
    x = np.asarray(inputs["x"], f64)
    w0 = np.asarray(inputs["w0"], f64)          # [8, 32, 8]
    w1 = np.asarray(inputs["w1"], f64)          # [1, 64, 32]
    a_src0 = np.asarray(inputs["a_src0"], f64)[..., 0]   # [8, 8]
    a_dst0 = np.asarray(inputs["a_dst0"], f64)[..., 0]   # [8, 8]
    a_src1 = np.asarray(inputs["a_src1"], f64)[0, :, 0]  # [32]
    a_dst1 = np.asarray(inputs["a_dst1"], f64)[0, :, 0]  # [32]
    b0 = np.asarray(inputs["b0"], f64)          # [8]
    b1 = np.asarray(inputs["b1"], f64)          # [32]

    al0 = np.asarray(inputs["bn0_gamma"], f64) / np.sqrt(
        np.asarray(inputs["bn0_var"], f64) + BN_EPS)
    sh0 = np.asarray(inputs["bn0_beta"], f64) - \
        np.asarray(inputs["bn0_mean"], f64) * al0
    al1 = np.asarray(inputs["bn1_gamma"], f64) / np.sqrt(
        np.asarray(inputs["bn1_var"], f64) + BN_EPS)
    sh1 = np.asarray(inputs["bn1_beta"], f64) - \
        np.asarray(inputs["bn1_mean"], f64) * al1

    # layer 0 folds
    w0flat = (al0[None, :, None] * w0).transpose(1, 0, 2).reshape(32, 64)
    beta0h = np.einsum("i,hio->ho", sh0, w0)     # [8, 8]
    beta0 = beta0h.reshape(64)
    as0 = al0[:, None] * np.einsum("hio,ho->ih", w0, a_src0)   # [32, 8]
    sb0 = np.einsum("ho,ho->h", beta0h, a_src0)
    ad0 = al0[:, None] * np.einsum("hio,ho->ih", w0, a_dst0)
    db0 = np.einsum("ho,ho->h", beta0h, a_dst0)

    w0aug = np.zeros((33, 72), f64)
    w0aug[0:32, 0:64] = w0flat
    w0aug[32, 0:64] = beta0
    w0aug[0:32, 64:72] = ad0
    w0aug[32, 64:72] = db0
    w0s = np.zeros((33, 8), f64)
    w0s[0:32, :] = as0
    w0s[32, :] = sb0

    # layer 1 folds
    w1m = w1[0]                                   # [64, 32]
    w1flat = al1[:, None] * w1m
    beta1 = sh1 @ w1m                             # [32]
    as1 = al1 * (w1m @ a_src1)
    sb1 = beta1 @ a_src1
    ad1 = al1 * (w1m @ a_dst1)
    db1 = beta1 @ a_dst1

    # per-chunk [16, 34] blocks: cols 0:32 w1, col 32 = ad1, col 33 pad
    w1ch = np.zeros((16, 4, 34), f64)
    for c in range(4):
        w1ch[:, c, 0:32] = w1flat[c * 16:(c + 1) * 16, :]
        w1ch[:, c, 32] = ad1[c * 16:(c + 1) * 16]
    w1bias = np.zeros((1, 34), f64)
    w1bias[0, 0:32] = beta1
    w1bias[0, 32] = db1

    w1sc = as1.reshape(4, 16).T                   # [16, 4]
    sb1t = np.array([[sb1]])

    sela = np.zeros((8, 8, 128), ml_dtypes.bfloat16)  # row h ones in block h
    for h in range(8):
        sela[h, h, :] = 1.0

    xT33 = np.zeros((33, N), np.float32)
    xT33[0:32, :] = x.T
    xT33[32, :] = 1.0

    return {
        "xT33_full": xT33,
        "w0aug": w0aug.astype(np.float32),
        "w0s": w0s.astype(np.float32),
        "w1ch": np.ascontiguousarray(w1ch.reshape(16, 4 * 34)
                                     ).astype(np.float32),
        "w1b": w1bias.astype(np.float32),
        "w1sc": w1sc.astype(np.float32),
        "sb1t": sb1t.astype(np.float32),
        "b0p": b0.reshape(8, 1).astype(np.float32),
        "nb0p": (-b0).reshape(8, 1).astype(np.float32),
        "b1f": b1.reshape(32, 1).astype(np.float32),
        "sela": sela.reshape(8, 8 * 128),
    }


def _in_maps(inputs):
    shared = _fold(inputs)
    xT33 = shared.pop("xT33_full")
    in_maps = []
    for c in range(NCORES):
        m = dict(shared)
        m["xT33"] = xT33
        m["xsT33"] = np.ascontiguousarray(xT33[:, c * RPC:(c + 1) * RPC])
        in_maps.append(m)
    return in_maps


def kernel(**inputs) -> np.ndarray:
    if "nc" not in _CACHE:
        _CACHE["nc"] = _build()
    nc = _CACHE["nc"]

    res = run_bass_kernel_spmd(nc, _in_maps(inputs), list(range(NCORES)))
    out = np.concatenate([res.results[c]["out"] for c in range(NCORES)],
                         axis=0)
    return out.astype(np.float32)


# revision 17
# speedup vs baseline: 1.2927x; 1.0505x over previous
"""GAT (2-layer dense-graph attention over 4096 nodes) as a Trainium2
Bass/Tile SPMD kernel across 8 NeuronCores.

Sharding: attention destination rows are sharded 512/core for both layers.
Each core computes the full source-side quantities (h', d) from the full x
and the s-scores only for its own 512 destination rows.

v2 design (from baseline trace analysis):
- No fp32-residual columns: pure-bf16 h' gives ~6e-4 rel err (gate 2e-2).
  Stationary per (jt, h) is 33 cols: h' at output partitions 0:7 (base 0),
  ones at 32 (base 32) -- engines can only address partition bases
  0/32/64/96, which pins those offsets.
- E' = max(e^{0.8 s_i} e^{d_j}, e^{0.2 d_j}) as one tensor_scalar per
  [128, 512] tile, split DVE (2/3) / GPSIMD (1/3) so tile production keeps
  pace with the PE and the HAM clock gate stays at 8/8 (2.4 GHz).
- x arrives host-pre-transposed (xT with bias row) -- no on-chip transposes.
- Per-chunk softmax-normalize + ELU is emitted interleaved into the NEXT
  head's tile stream so the in-order DVE queue never head-blocks the PE.
- Layer-1 projection h'1 = elu_h @ W1 runs LOCALLY on the owning core,
  PSUM-accumulated chunk-by-chunk during the L0 heads. After head 7 a
  single bf16 [512, 36] payload {h'1, ones, e^{d1}, e^{0.2 d1}} is
  AllGathered (vs 4 fp32 gathers + serial remote projection in v1).
- A dummy 1KB AllGather at kernel start absorbs the ~11 us
  first-collective setup penalty.
"""

import numpy as np
import ml_dtypes

import concourse.bacc as bacc
import concourse.mybir as mybir
import concourse.tile as tile
from concourse import masks
from concourse.bass_utils import run_bass_kernel_spmd

F32 = mybir.dt.float32
BF16 = mybir.dt.bfloat16
N = 4096
NCORES = 8
RPC = N // NCORES          # destination rows per core = 512
NJT = N // 128             # 32 j-tiles of 128 source rows
BN_EPS = 1e-5

_CACHE = {}
DEBUG = False


def _build():
    nc = bacc.Bacc("TRN2", target_bir_lowering=False, debug=False,
                   num_devices=NCORES)

    xT_d = nc.dram_tensor("xT33", [33, N], F32, kind="ExternalInput")
    xsT_d = nc.dram_tensor("xsT33", [33, RPC], F32, kind="ExternalInput")
    w0aug_d = nc.dram_tensor("w0aug", [33, 72], F32, kind="ExternalInput")
    w0s_d = nc.dram_tensor("w0s", [33, 8], F32, kind="ExternalInput")
    w1ch_d = nc.dram_tensor("w1ch", [16, 4 * 34], F32, kind="ExternalInput")
    w1b_d = nc.dram_tensor("w1b", [1, 34], F32, kind="ExternalInput")
    w1sc_d = nc.dram_tensor("w1sc", [16, 4], F32, kind="ExternalInput")
    sb1_d = nc.dram_tensor("sb1t", [1, 1], F32, kind="ExternalInput")
    b0p_d = nc.dram_tensor("b0p", [8, 1], F32, kind="ExternalInput")
    nb0p_d = nc.dram_tensor("nb0p", [8, 1], F32, kind="ExternalInput")
    b1_d = nc.dram_tensor("b1f", [32, 1], F32, kind="ExternalInput")
    sela_d = nc.dram_tensor("sela", [8, 8 * 128], BF16, kind="ExternalInput")
    out_d = nc.dram_tensor("out", [RPC, 32], F32, kind="ExternalOutput")
    if DEBUG:
        dbg_a0_d = nc.dram_tensor("dbg_a0", [8, 512], BF16,
                                  kind="ExternalOutput")
        dbg_ds_d = nc.dram_tensor("dbg_ds", [128, 8], F32,
                                  kind="ExternalOutput")
        dbg_hpa_d = nc.dram_tensor("dbg_hpa", [128, 34], BF16,
                                   kind="ExternalOutput")
        dbg_cont_d = nc.dram_tensor("dbg_cont", [16, 4 * 512], F32,
                                    kind="ExternalOutput")
        dbg_pay_d = nc.dram_tensor("dbg_pay", [36, 512], BF16,
                                   kind="ExternalOutput")
        dbg_hpa1_d = nc.dram_tensor("dbg_hpa1", [128, 36], BF16,
                                    kind="ExternalOutput")
        dbg_a1_d = nc.dram_tensor("dbg_a1", [1, 512], BF16,
                                  kind="ExternalOutput")
        dbg_pg1_d = nc.dram_tensor("dbg_pg1", [33, 512], F32,
                                   kind="ExternalOutput")

    with tile.TileContext(nc) as tc:
        with (
            tc.tile_pool(name="const", bufs=1) as const,
            tc.tile_pool(name="persist", bufs=1) as per,
            tc.tile_pool(name="pacc", bufs=1, space="PSUM") as pacc,
            tc.tile_pool(name="dram", bufs=1, space="DRAM") as dram,
        ):
            wsrc = const.tile([128, 512], BF16)
            nc.vector.memset(wsrc[:], 0.5)
            wlhs = const.tile([128, 128], BF16)
            nc.vector.memset(wlhs[:], 0.25)
            ident = const.tile([128, 128], F32)
            masks.make_identity(nc, ident[:])
            identB = const.tile([36, 36], BF16)
            nc.vector.tensor_copy(identB[:], ident[0:36, 0:36])
            ones8 = const.tile([1, 8], F32)
            nc.vector.memset(ones8[:], 1.0)
            ones32 = const.tile([1, 32], F32)
            nc.vector.memset(ones32[:], 1.0)
            ones512 = const.tile([1, 512], F32)
            nc.vector.memset(ones512[:], 1.0)
            ones_row_bf = const.tile([1, 128], BF16)
            nc.vector.memset(ones_row_bf[:], 1.0)
            sela = const.tile([8, 8 * 128], BF16)
            nc.sync.dma_start(sela[:], sela_d[:])

            w0aug = const.tile([33, 72], F32)
            nc.sync.dma_start(w0aug[:], w0aug_d[:])
            w0s = const.tile([33, 8], F32)
            nc.sync.dma_start(w0s[:], w0s_d[:])
            w1ch = const.tile([16, 4, 34], F32)
            nc.sync.dma_start(w1ch[:], w1ch_d[:].rearrange("p (c f) -> p c f", c=4))
            w1b = const.tile([1, 34], F32)
            nc.sync.dma_start(w1b[:], w1b_d[:])
            w1sc = const.tile([16, 4], F32)
            nc.sync.dma_start(w1sc[:], w1sc_d[:])
            sb1t = const.tile([1, 1], F32)
            nc.sync.dma_start(sb1t[:], sb1_d[:])
            b0p = const.tile([8, 1], F32)
            nc.sync.dma_start(b0p[:], b0p_d[:])
            nb0p = const.tile([8, 1], F32)
            nc.sync.dma_start(nb0p[:], nb0p_d[:])
            b1c = const.tile([32, 1], F32)
            nc.sync.dma_start(b1c[:], b1_d[:])

            # big persistent sbuf tensors
            xT = per.tile([33, N], F32)        # x^T plus ones row (from host)
            xsT = per.tile([33, RPC], F32)
            # stationary per (jt, h): h' bf16 at cols 0:8, ones col at 32
            hpa0 = per.tile([128, NJT, 8, 34], BF16)
            dstage = per.tile([128, NJT, 8], F32)    # raw d0 per (j, jt, h)
            d0e = per.tile([128, NJT, 8], F32)       # e^{d0}
            d0e2 = per.tile([128, NJT, 8], F32)      # e^{0.2 d0}
            atile = per.tile([128, 8, 512], BF16)    # e^{0.8 s0} bcast
            a0row = per.tile([8, 512], BF16)
            contc = per.tile([16, 4, 512], F32)      # elu(out0)^T per chunk
            # layer 1 stationary: h'1 bf16 0:32, ones 32, e^{d1} 33,
            # e^{0.2 d1} 34, pad 35
            hpa1 = per.tile([128, NJT, 36], BF16)
            d1e = per.tile([128, NJT, 1], F32)
            d1e2 = per.tile([128, NJT, 1], F32)
            a1tile = per.tile([128, 512], BF16)
            a1row = per.tile([1, 512], BF16)
            paySrc = per.tile([36, 512], BF16)
            payT = per.tile([128, 4, 36], BF16)
            ed1st = per.tile([1, 512], BF16)
            ed2st = per.tile([1, 512], BF16)
            o1s = per.tile([32, 512], F32)

            # PSUM accumulators alive across the whole heads phase
            p1T = pacc.tile([33, 512], F32, tag="p1T")   # local h'1^T (+d1)
            ps1 = pacc.tile([1, 512], F32, tag="ps1")    # local s1

            dumin = dram.tile([8, 16], F32, name="dumin", tag="dumin")
            dumout = dram.tile([NCORES * 8, 16], F32, name="dumout",
                               tag="dumout")
            contd1 = dram.tile([RPC, 36], BF16, name="contd1", tag="contd1")
            agc1 = dram.tile([N, 36], BF16, name="agc1", tag="agc1")

            # ---------------- Phase A: projections -----------------
            with (
                tc.tile_pool(name="ld", bufs=2) as ld,
                tc.tile_pool(name="mm80", bufs=2, space="PSUM") as mm80,
                tc.tile_pool(name="pssa0", bufs=1, space="PSUM") as pssa0,
                tc.tile_pool(name="pssa", bufs=2, space="PSUM") as pssa,
                tc.tile_pool(name="wp", bufs=1, space="PSUM") as wp,
            ):
                # PE warm-up burst: back-to-back matmuls flip the HAM clock
                # gate to 8/8 while input DMAs are still in flight
                wps = wp.tile([128, 512], F32)
                for r in range(20):
                    nc.tensor.matmul(wps[:], wlhs[:], wsrc[:],
                                     start=(r == 0), stop=(r == 19))

                # dummy collective to absorb first-CC setup cost (overlaps A)
                dustage = ld.tile([8, 16], F32, tag="dustage")
                nc.vector.memset(dustage[:], 1.0)
                nc.sync.dma_start(dumin[:], dustage[:])
                nc.gpsimd.collective_compute(
                    "AllGather",
                    mybir.AluOpType.bypass,
                    replica_groups=[list(range(NCORES))],
                    ins=[dumin.opt()],
                    outs=[dumout.opt()],
                )

                nc.sync.dma_start(xT[:], xT_d[:])
                nc.sync.dma_start(xsT[:], xsT_d[:])

                # zero-init big stationaries (ones cols set below)
                nc.gpsimd.memset(hpa0[:], 0.0)
                nc.vector.memset(hpa0[:, :, :, 32:33], 1.0)
                nc.gpsimd.memset(hpa1[:], 0.0)
                nc.gpsimd.memset(paySrc[:], 0.0)

                # s0 rows for this core's 512 dst rows; a = e^{0.8 s}
                ps0 = pssa0.tile([8, 512], F32, tag="ps0")
                nc.tensor.matmul(ps0[:], w0s[:], xsT[:])
                nc.scalar.activation(a0row[:], ps0[:],
                                     mybir.ActivationFunctionType.Exp,
                                     scale=0.8)
                for h in range(8):
                    pa = pssa.tile([128, 512], F32, tag="pa")
                    nc.tensor.matmul(pa[:], sela[:, h * 128:(h + 1) * 128],
                                     a0row[:])
                    if h % 2 == 0:
                        nc.vector.tensor_copy(atile[:, h, :], pa[:])
                    else:
                        nc.scalar.copy(atile[:, h, :], pa[:])

                # h'0 (bf16) and raw d0 per j-tile
                for jt in range(NJT):
                    p80 = mm80.tile([128, 72], F32)
                    nc.tensor.matmul(p80[:], xT[:, jt * 128:(jt + 1) * 128],
                                     w0aug[:])
                    hsrc = p80[:, 0:64].rearrange("p (h o) -> p h o", h=8)
                    nc.scalar.copy(hpa0[:, jt, :, 0:8], hsrc)
                    nc.vector.tensor_copy(dstage[:, jt, :], p80[:, 64:72])
                    if jt % 8 == 7:
                        b = jt - 7
                        nc.scalar.activation(
                            d0e[:, b:jt + 1, :], dstage[:, b:jt + 1, :],
                            mybir.ActivationFunctionType.Exp)
                        nc.scalar.activation(
                            d0e2[:, b:jt + 1, :], dstage[:, b:jt + 1, :],
                            mybir.ActivationFunctionType.Exp, scale=0.2)

            # ------- Phase B: layer-0 attention + local L1 projection -------
            with (
                tc.tile_pool(name="epool", bufs=14) as epool,
                tc.tile_pool(name="agg", bufs=3, space="PSUM") as agg,
                tc.tile_pool(name="prbp", bufs=2, space="PSUM") as prbp,
                tc.tile_pool(name="tmp", bufs=3) as tmp,
            ):
                pgs = {}

                def emit_head_tile(h, jt):
                    if jt == 0:
                        pgs[h] = agg.tile([33, 512], F32, name=f"pg{h}",
                                          tag="pg")
                    pg = pgs[h]
                    e = epool.tile([128, 512], BF16, tag="e")
                    nc.vector.tensor_scalar(
                        e[:], atile[:, h, :],
                        d0e[:, jt, h:h + 1], d0e2[:, jt, h:h + 1],
                        op0=mybir.AluOpType.mult,
                        op1=mybir.AluOpType.max)
                    nc.tensor.matmul(pg[:], hpa0[:, jt, h, 0:33], e[:],
                                     start=(jt == 0), stop=(jt == NJT - 1))

                def emit_norm(h, step):
                    """Normalize + bias + ELU for head h, split into 6 steps
                    so it interleaves with the next head's tile stream."""
                    ch, hh = h // 2, h % 2
                    pg = pgs[h]
                    st = norm_state[h]
                    if step == 0:
                        # 1/x = exp(-ln(x)) on the scalar engine: keeps the
                        # DVE free (vector.reciprocal costs 3.3us and
                        # head-blocks e-tile production)
                        st['lnd'] = tmp.tile([1, 512], F32, name="lnd",
                                             tag="lnd")
                        nc.scalar.activation(st['lnd'][:], pg[32:33, :],
                                             mybir.ActivationFunctionType.Ln)
                        st['rcp'] = tmp.tile([1, 512], F32, name="rcp",
                                             tag="rcp")
                        nc.scalar.activation(st['rcp'][:], st['lnd'][:],
                                             mybir.ActivationFunctionType.Exp,
                                             scale=-1.0)
                    elif step == 1:
                        prb = prbp.tile([8, 512], F32)
                        nc.tensor.matmul(prb[:], ones8[:], st['rcp'][:])
                        st['prbs'] = tmp.tile([8, 512], F32, name="prbs", tag="prbs")
                        nc.scalar.copy(st['prbs'][:], prb[:])
                    elif step == 2:
                        st['nrm'] = tmp.tile([8, 512], F32, name="nrm", tag="nrm")
                        nc.vector.tensor_tensor(st['nrm'][:], pg[0:8, :],
                                                st['prbs'][:],
                                                op=mybir.AluOpType.mult)
                    elif step == 3:
                        # eneg = exp(min(nrm+b0, 0)) = exp(-relu(-nrm-b0))
                        st['mneg'] = tmp.tile([8, 512], F32, name="mneg", tag="mneg")
                        nc.scalar.activation(
                            st['mneg'][:], st['nrm'][:],
                            mybir.ActivationFunctionType.Relu,
                            bias=nb0p[:], scale=-1.0)
                        st['eneg'] = tmp.tile([8, 512], F32, name="eneg", tag="eneg")
                        nc.scalar.activation(
                            st['eneg'][:], st['mneg'][:],
                            mybir.ActivationFunctionType.Exp, scale=-1.0)
                    elif step == 4:
                        st['ppos'] = tmp.tile([8, 512], F32, name="ppos", tag="ppos")
                        nc.scalar.activation(
                            st['ppos'][:], st['nrm'][:],
                            mybir.ActivationFunctionType.Relu,
                            bias=b0p[:])
                    elif step == 5:
                        # elu half-row = (eneg - 1) + ppos
                        st['half'] = tmp.tile([8, 512], F32, name="half", tag="half")
                        nc.vector.scalar_tensor_tensor(
                            st['half'][:], st['eneg'][:], -1.0, st['ppos'][:],
                            op0=mybir.AluOpType.add,
                            op1=mybir.AluOpType.add)
                        nc.sync.dma_start(contc[hh * 8:(hh + 1) * 8, ch, :],
                                          st['half'][:])
                        if hh == 1:
                            # chunk complete: accumulate local L1 projection
                            # h'1^T += W1_ch^T @ contc_ch  and s1 partials
                            nc.tensor.matmul(p1T[:], w1ch[:, ch, 0:33],
                                             contc[:, ch, :],
                                             start=(ch == 0), stop=False)
                            nc.tensor.matmul(ps1[:], w1sc[:, ch:ch + 1],
                                             contc[:, ch, :],
                                             start=(ch == 0), stop=False)

                norm_state = [dict() for _ in range(8)]
                NORM_AT = [7, 11, 15, 19, 23, 27]  # jt positions in next head
                for h in range(8):
                    for jt in range(NJT):
                        emit_head_tile(h, jt)
                        if h > 0 and jt in NORM_AT:
                            emit_norm(h - 1, NORM_AT.index(jt))
                # head 7 norm: nothing left to interleave with
                for step in range(6):
                    emit_norm(7, step)

            # ---------------- Phase C: payload + gather ----------------
            with (
                tc.tile_pool(name="pd", bufs=2, space="PSUM") as pd,
                tc.tile_pool(name="tp2", bufs=2, space="PSUM") as tp2,
                tc.tile_pool(name="ot", bufs=2) as ot,
            ):
                # close the local L1 projection: bias row (beta1, db1) and
                # s1 bias, then exps
                nc.tensor.matmul(p1T[:], w1b[:, 0:33], ones512[:],
                                 start=False, stop=True)
                nc.tensor.matmul(ps1[:], sb1t[:], ones512[:],
                                 start=False, stop=True)
                nc.scalar.activation(a1row[:], ps1[:],
                                     mybir.ActivationFunctionType.Exp,
                                     scale=0.8)
                pa1 = pd.tile([128, 512], F32, tag="pa1")
                nc.tensor.matmul(pa1[:], ones_row_bf[:], a1row[:])
                nc.vector.tensor_copy(a1tile[:], pa1[:])

                # payload rows: 0:32 h'1 bf16, 32 ones, 33 e^{d1},
                # 34 e^{0.2 d1}, 35 pad
                nc.scalar.copy(paySrc[0:32, :], p1T[0:32, :])
                nc.vector.memset(paySrc[32:33, :], 1.0)
                # engine writes are limited to partition bases 0/32/64/96:
                # stage the exps at base 0 and DMA them into rows 33/34
                nc.scalar.activation(ed1st[:], p1T[32:33, :],
                                     mybir.ActivationFunctionType.Exp)
                nc.scalar.activation(ed2st[:], p1T[32:33, :],
                                     mybir.ActivationFunctionType.Exp,
                                     scale=0.2)
                nc.sync.dma_start(paySrc[33:34, :], ed1st[:])
                nc.sync.dma_start(paySrc[34:35, :], ed2st[:])

                if DEBUG:
                    nc.sync.dma_start(dbg_a0_d[:], a0row[:])
                    nc.sync.dma_start(dbg_ds_d[:], dstage[:, 0, :])
                    nc.sync.dma_start(dbg_hpa_d[:], hpa0[:, 0, 0, :])
                    nc.sync.dma_start(
                        dbg_cont_d[:],
                        contc[:].rearrange("p c f -> p (c f)"))
                    nc.sync.dma_start(dbg_pay_d[:], paySrc[:])
                for k in range(4):
                    pt = tp2.tile([128, 36], BF16)
                    nc.tensor.matmul(pt[:],
                                     paySrc[:, k * 128:(k + 1) * 128],
                                     identB[:], is_transpose=True)
                    nc.scalar.copy(payT[:, k, :], pt[:])
                nc.sync.dma_start(
                    contd1[:].rearrange("(k p) c -> p k c", p=128),
                    payT[:])
                nc.gpsimd.collective_compute(
                    "AllGather",
                    mybir.AluOpType.bypass,
                    replica_groups=[list(range(NCORES))],
                    ins=[contd1.opt()],
                    outs=[agc1.opt()],
                )
                nc.sync.dma_start(
                    hpa1[:],
                    agc1[:].rearrange("(k p) c -> p k c", p=128))
                nc.scalar.copy(d1e[:], hpa1[:, :, 33:34])
                nc.scalar.copy(d1e2[:], hpa1[:, :, 34:35])

            # ---------------- Phase D: layer-1 attention ----------------
            with (
                tc.tile_pool(name="e1pool", bufs=10) as e1pool,
                tc.tile_pool(name="agg1", bufs=1, space="PSUM") as agg1,
                tc.tile_pool(name="pd2", bufs=2, space="PSUM") as pd2,
                tc.tile_pool(name="tp3", bufs=2, space="PSUM") as tp3,
                tc.tile_pool(name="ot2", bufs=2) as ot2,
                tc.tile_pool(name="tmp2", bufs=2) as tmp2,
            ):
                pg1 = agg1.tile([33, 512], F32)
                for jt in range(NJT):
                    e1 = e1pool.tile([128, 512], BF16, tag="e1")
                    nc.vector.tensor_scalar(
                        e1[:], a1tile[:],
                        d1e[:, jt, :], d1e2[:, jt, :],
                        op0=mybir.AluOpType.mult,
                        op1=mybir.AluOpType.max)
                    nc.tensor.matmul(pg1[:], hpa1[:, jt, 0:33], e1[:],
                                     start=(jt == 0), stop=(jt == NJT - 1))

                if DEBUG:
                    nc.sync.dma_start(dbg_hpa1_d[:], hpa1[:, 0, :])
                    nc.sync.dma_start(dbg_a1_d[:], a1row[:])
                    pg1s = tmp2.tile([33, 512], F32, tag="pg1s")
                    nc.vector.tensor_copy(pg1s[:], pg1[:])
                    nc.sync.dma_start(dbg_pg1_d[:], pg1s[:])
                lnd1 = tmp2.tile([1, 512], F32, tag="lnd1")
                nc.scalar.activation(lnd1[:], pg1[32:33, :],
                                     mybir.ActivationFunctionType.Ln)
                rcp1 = tmp2.tile([1, 512], F32, tag="rcp1")
                nc.scalar.activation(rcp1[:], lnd1[:],
                                     mybir.ActivationFunctionType.Exp,
                                     scale=-1.0)
                prb1 = pd2.tile([32, 512], F32, tag="prb1")
                nc.tensor.matmul(prb1[:], ones32[:], rcp1[:])
                prbs1 = tmp2.tile([32, 512], F32, tag="prbs1")
                nc.scalar.copy(prbs1[:], prb1[:])
                nc.vector.tensor_tensor(o1s[:], pg1[0:32, :], prbs1[:],
                                        op=mybir.AluOpType.mult)
                nc.vector.tensor_scalar_add(o1s[:], o1s[:], b1c[:])

                for ic in range(4):
                    pt2 = tp3.tile([128, 32], F32)
                    nc.tensor.matmul(pt2[:],
                                     o1s[:, ic * 128:(ic + 1) * 128],
                                     ident[0:32, 0:32], is_transpose=True)
                    ob = ot2.tile([128, 32], F32, tag="ob")
                    nc.scalar.copy(ob[:], pt2[:])
                    nc.sync.dma_start(out_d[ic * 128:(ic + 1) * 128, :],
                                      ob[:])

    nc.compile()
    return nc


def _fold(inputs):
    """Host-side BN folding, attention-projection folding, x transpose."""
    f64 = np.float64
    x = np.asarray(inputs["x"], f64)
    w0 = np.asarray(inputs["w0"], f64)          # [8, 32, 8]
    w1 = np.asarray(inputs["w1"], f64)          # [1, 64, 32]
    a_src0 = np.asarray(inputs["a_src0"], f64)[..., 0]   # [8, 8]
    a_dst0 = np.asarray(inputs["a_dst0"], f64)[..., 0]   # [8, 8]
    a_src1 = np.asarray(inputs["a_src1"], f64)[0, :, 0]  # [32]
    a_dst1 = np.asarray(inputs["a_dst1"], f64)[0, :, 0]  # [32]
    b0 = np.asarray(inputs["b0"], f64)          # [8]
    b1 = np.asarray(inputs["b1"], f64)          # [32]

    al0 = np.asarray(inputs["bn0_gamma"], f64) / np.sqrt(
        np.asarray(inputs["bn0_var"], f64) + BN_EPS)
    sh0 = np.asarray(inputs["bn0_beta"], f64) - \
        np.asarray(inputs["bn0_mean"], f64) * al0
    al1 = np.asarray(inputs["bn1_gamma"], f64) / np.sqrt(
        np.asarray(inputs["bn1_var"], f64) + BN_EPS)
    sh1 = np.asarray(inputs["bn1_beta"], f64) - \
        np.asarray(inputs["bn1_mean"], f64) * al1

    # layer 0 folds
    w0flat = (al0[None, :, None] * w0).transpose(1, 0, 2).reshape(32, 64)
    beta0h = np.einsum("i,hio->ho", sh0, w0)     # [8, 8]
    beta0 = beta0h.reshape(64)
    as0 = al0[:, None] * np.einsum("hio,ho->ih", w0, a_src0)   # [32, 8]
    sb0 = np.einsum("ho,ho->h", beta0h, a_src0)
    ad0 = al0[:, None] * np.einsum("hio,ho->ih", w0, a_dst0)
    db0 = np.einsum("ho,ho->h", beta0h, a_dst0)

    w0aug = np.zeros((33, 72), f64)
    w0aug[0:32, 0:64] = w0flat
    w0aug[32, 0:64] = beta0
    w0aug[0:32, 64:72] = ad0
    w0aug[32, 64:72] = db0
    w0s = np.zeros((33, 8), f64)
    w0s[0:32, :] = as0
    w0s[32, :] = sb0

    # layer 1 folds
    w1m = w1[0]                                   # [64, 32]
    w1flat = al1[:, None] * w1m
    beta1 = sh1 @ w1m                             # [32]
    as1 = al1 * (w1m @ a_src1)
    sb1 = beta1 @ a_src1
    ad1 = al1 * (w1m @ a_dst1)
    db1 = beta1 @ a_dst1

    # per-chunk [16, 34] blocks: cols 0:32 w1, col 32 = ad1, col 33 pad
    w1ch = np.zeros((16, 4, 34), f64)
    for c in range(4):
        w1ch[:, c, 0:32] = w1flat[c * 16:(c + 1) * 16, :]
        w1ch[:, c, 32] = ad1[c * 16:(c + 1) * 16]
    w1bias = np.zeros((1, 34), f64)
    w1bias[0, 0:32] = beta1
    w1bias[0, 32] = db1

    w1sc = as1.reshape(4, 16).T                   # [16, 4]
    sb1t = np.array([[sb1]])

    sela = np.zeros((8, 8, 128), ml_dtypes.bfloat16)  # row h ones in block h
    for h in range(8):
        sela[h, h, :] = 1.0

    xT33 = np.zeros((33, N), np.float32)
    xT33[0:32, :] = x.T
    xT33[32, :] = 1.0

    return {
        "xT33_full": xT33,
        "w0aug": w0aug.astype(np.float32),
        "w0s": w0s.astype(np.float32),
        "w1ch": np.ascontiguousarray(w1ch.reshape(16, 4 * 34)
                                     ).astype(np.float32),
        "w1b": w1bias.astype(np.float32),
        "w1sc": w1sc.astype(np.float32),
        "sb1t": sb1t.astype(np.float32),
        "b0p": b0.reshape(8, 1).astype(np.float32),
        "nb0p": (-b0).reshape(8, 1).astype(np.float32),
        "b1f": b1.reshape(32, 1).astype(np.float32),
        "sela": sela.reshape(8, 8 * 128),
    }


def _in_maps(inputs):
    shared = _fold(inputs)
    xT33 = shared.pop("xT33_full")
    in_maps = []
    for c in range(NCORES):
        m = dict(shared)
        m["xT33"] = xT33
        m["xsT33"] = np.ascontiguousarray(xT33[:, c * RPC:(c + 1) * RPC])
        in_maps.append(m)
    return in_maps


def kernel(**inputs) -> np.ndarray:
    if "nc" not in _CACHE:
        _CACHE["nc"] = _build()
    nc = _CACHE["nc"]

    res = run_bass_kernel_spmd(nc, _in_maps(inputs), list(range(NCORES)))
    out = np.concatenate([res.results[c]["out"] for c in range(NCORES)],
                         axis=0)
    return out.astype(np.float32)


# revision 18
# speedup vs baseline: 1.3549x; 1.0481x over previous
"""GAT (2-layer dense-graph attention over 4096 nodes) as a Trainium2
Bass/Tile SPMD kernel across 8 NeuronCores.

Sharding: attention destination rows are sharded 512/core for both layers.
Each core computes the full source-side quantities (h', d) from the full x
and the s-scores only for its own 512 destination rows.

v2 design (from baseline trace analysis):
- No fp32-residual columns: pure-bf16 h' gives ~6e-4 rel err (gate 2e-2).
  Stationary per (jt, h) is 33 cols: h' at output partitions 0:7 (base 0),
  ones at 32 (base 32) -- engines can only address partition bases
  0/32/64/96, which pins those offsets.
- E' = max(e^{0.8 s_i} e^{d_j}, e^{0.2 d_j}) as one tensor_scalar per
  [128, 512] tile, split DVE (2/3) / GPSIMD (1/3) so tile production keeps
  pace with the PE and the HAM clock gate stays at 8/8 (2.4 GHz).
- x arrives host-pre-transposed (xT with bias row) -- no on-chip transposes.
- Per-chunk softmax-normalize + ELU is emitted interleaved into the NEXT
  head's tile stream so the in-order DVE queue never head-blocks the PE.
- Layer-1 projection h'1 = elu_h @ W1 runs LOCALLY on the owning core,
  PSUM-accumulated chunk-by-chunk during the L0 heads. After head 7 a
  single bf16 [512, 36] payload {h'1, ones, e^{d1}, e^{0.2 d1}} is
  AllGathered (vs 4 fp32 gathers + serial remote projection in v1).
- A dummy 1KB AllGather at kernel start absorbs the ~11 us
  first-collective setup penalty.
"""

import numpy as np
import ml_dtypes

import concourse.bacc as bacc
import concourse.mybir as mybir
import concourse.tile as tile
from concourse import masks
from concourse.bass_utils import run_bass_kernel_spmd

F32 = mybir.dt.float32
BF16 = mybir.dt.bfloat16
N = 4096
NCORES = 8
RPC = N // NCORES          # destination rows per core = 512
NJT = N // 128             # 32 j-tiles of 128 source rows
BN_EPS = 1e-5

_CACHE = {}
DEBUG = False


def _build():
    nc = bacc.Bacc("TRN2", target_bir_lowering=False, debug=False,
                   num_devices=NCORES)

    xT_d = nc.dram_tensor("xT33", [33, N], F32, kind="ExternalInput")
    xsT_d = nc.dram_tensor("xsT33", [33, RPC], F32, kind="ExternalInput")
    w0aug_d = nc.dram_tensor("w0aug", [33, 72], F32, kind="ExternalInput")
    w0s_d = nc.dram_tensor("w0s", [33, 8], F32, kind="ExternalInput")
    w1ch_d = nc.dram_tensor("w1ch", [16, 4 * 34], F32, kind="ExternalInput")
    w1b_d = nc.dram_tensor("w1b", [1, 34], F32, kind="ExternalInput")
    w1sc_d = nc.dram_tensor("w1sc", [16, 4], F32, kind="ExternalInput")
    sb1_d = nc.dram_tensor("sb1t", [1, 1], F32, kind="ExternalInput")
    b0p_d = nc.dram_tensor("b0p", [8, 1], F32, kind="ExternalInput")
    nb0p_d = nc.dram_tensor("nb0p", [8, 1], F32, kind="ExternalInput")
    b1_d = nc.dram_tensor("b1f", [32, 1], F32, kind="ExternalInput")
    sela_d = nc.dram_tensor("sela", [8, 8 * 128], BF16, kind="ExternalInput")
    out_d = nc.dram_tensor("out", [RPC, 32], F32, kind="ExternalOutput")
    if DEBUG:
        dbg_a0_d = nc.dram_tensor("dbg_a0", [8, 512], BF16,
                                  kind="ExternalOutput")
        dbg_ds_d = nc.dram_tensor("dbg_ds", [128, 8], F32,
                                  kind="ExternalOutput")
        dbg_hpa_d = nc.dram_tensor("dbg_hpa", [128, 34], BF16,
                                   kind="ExternalOutput")
        dbg_cont_d = nc.dram_tensor("dbg_cont", [16, 4 * 512], F32,
                                    kind="ExternalOutput")
        dbg_pay_d = nc.dram_tensor("dbg_pay", [36, 512], BF16,
                                   kind="ExternalOutput")
        dbg_hpa1_d = nc.dram_tensor("dbg_hpa1", [128, 36], BF16,
                                    kind="ExternalOutput")
        dbg_a1_d = nc.dram_tensor("dbg_a1", [1, 512], BF16,
                                  kind="ExternalOutput")
        dbg_pg1_d = nc.dram_tensor("dbg_pg1", [33, 512], F32,
                                   kind="ExternalOutput")

    with tile.TileContext(nc) as tc:
        with (
            tc.tile_pool(name="const", bufs=1) as const,
            tc.tile_pool(name="persist", bufs=1) as per,
            tc.tile_pool(name="pacc", bufs=1, space="PSUM") as pacc,
            tc.tile_pool(name="dram", bufs=1, space="DRAM") as dram,
        ):
            wsrc = const.tile([128, 512], BF16)
            nc.vector.memset(wsrc[:], 0.5)
            wlhs = const.tile([128, 128], BF16)
            nc.vector.memset(wlhs[:], 0.25)
            ident = const.tile([128, 128], F32)
            masks.make_identity(nc, ident[:])
            identB = const.tile([36, 36], BF16)
            nc.vector.tensor_copy(identB[:], ident[0:36, 0:36])
            ones8 = const.tile([1, 8], F32)
            nc.vector.memset(ones8[:], 1.0)
            ones32 = const.tile([1, 32], F32)
            nc.vector.memset(ones32[:], 1.0)
            ones512 = const.tile([1, 512], F32)
            nc.vector.memset(ones512[:], 1.0)
            ones_row_bf = const.tile([1, 128], BF16)
            nc.vector.memset(ones_row_bf[:], 1.0)
            sela = const.tile([8, 8 * 128], BF16)
            nc.sync.dma_start(sela[:], sela_d[:])

            w0aug = const.tile([33, 72], F32)
            nc.sync.dma_start(w0aug[:], w0aug_d[:])
            w0s = const.tile([33, 8], F32)
            nc.sync.dma_start(w0s[:], w0s_d[:])
            w1ch = const.tile([16, 4, 34], F32)
            nc.sync.dma_start(w1ch[:], w1ch_d[:].rearrange("p (c f) -> p c f", c=4))
            w1b = const.tile([1, 34], F32)
            nc.sync.dma_start(w1b[:], w1b_d[:])
            w1sc = const.tile([16, 4], F32)
            nc.sync.dma_start(w1sc[:], w1sc_d[:])
            sb1t = const.tile([1, 1], F32)
            nc.sync.dma_start(sb1t[:], sb1_d[:])
            b0p = const.tile([8, 1], F32)
            nc.sync.dma_start(b0p[:], b0p_d[:])
            nb0p = const.tile([8, 1], F32)
            nc.sync.dma_start(nb0p[:], nb0p_d[:])
            b1c = const.tile([32, 1], F32)
            nc.sync.dma_start(b1c[:], b1_d[:])

            # big persistent sbuf tensors
            xT = per.tile([33, N], F32)        # x^T plus ones row (from host)
            xsT = per.tile([33, RPC], F32)
            # stationary per (jt, h): h' bf16 at cols 0:8, ones col at 32
            hpa0 = per.tile([128, NJT, 8, 34], BF16)
            dstage = per.tile([128, NJT, 8], F32)    # raw d0 per (j, jt, h)
            d0e = per.tile([128, NJT, 8], F32)       # e^{d0}
            d0e2 = per.tile([128, NJT, 8], F32)      # e^{0.2 d0}
            atile = per.tile([128, 8, 512], BF16)    # e^{0.8 s0} bcast
            a0row = per.tile([8, 512], BF16)
            contc = per.tile([16, 4, 512], F32)      # elu(out0)^T per chunk
            # layer 1 stationary: h'1 bf16 0:32, ones 32, e^{d1} 33,
            # e^{0.2 d1} 34, pad 35
            hpa1 = per.tile([128, NJT, 36], BF16)
            d1e = per.tile([128, NJT, 1], F32)
            d1e2 = per.tile([128, NJT, 1], F32)
            a1tile = per.tile([128, 512], BF16)
            a1row = per.tile([1, 512], BF16)
            paySrc = per.tile([36, 512], BF16)
            payT = per.tile([128, 4, 36], BF16)
            ed1st = per.tile([1, 512], BF16)
            ed2st = per.tile([1, 512], BF16)
            o1s = per.tile([32, 512], F32)

            # PSUM accumulators alive across the whole heads phase
            p1T = pacc.tile([33, 512], F32, tag="p1T")   # local h'1^T (+d1)
            ps1 = pacc.tile([1, 512], F32, tag="ps1")    # local s1

            dumin = dram.tile([8, 16], F32, name="dumin", tag="dumin")
            dumout = dram.tile([NCORES * 8, 16], F32, name="dumout",
                               tag="dumout")
            contd1 = dram.tile([RPC, 36], BF16, name="contd1", tag="contd1")
            agc1 = dram.tile([N, 36], BF16, name="agc1", tag="agc1")

            # ---------------- Phase A: projections -----------------
            with (
                tc.tile_pool(name="ld", bufs=2) as ld,
                tc.tile_pool(name="mm80", bufs=2, space="PSUM") as mm80,
                tc.tile_pool(name="pssa0", bufs=1, space="PSUM") as pssa0,
                tc.tile_pool(name="pssa", bufs=2, space="PSUM") as pssa,
                tc.tile_pool(name="wp", bufs=1, space="PSUM") as wp,
            ):
                # PE warm-up burst: back-to-back matmuls flip the HAM clock
                # gate to 8/8 while input DMAs are still in flight
                wps = wp.tile([128, 512], F32)
                for r in range(20):
                    nc.tensor.matmul(wps[:], wlhs[:], wsrc[:],
                                     start=(r == 0), stop=(r == 19))

                # dummy collective to absorb first-CC setup cost (overlaps A)
                dustage = ld.tile([8, 16], F32, tag="dustage")
                nc.vector.memset(dustage[:], 1.0)
                nc.sync.dma_start(dumin[:], dustage[:])
                nc.gpsimd.collective_compute(
                    "AllGather",
                    mybir.AluOpType.bypass,
                    replica_groups=[list(range(NCORES))],
                    ins=[dumin.opt()],
                    outs=[dumout.opt()],
                )

                nc.sync.dma_start(xT[:], xT_d[:])
                nc.sync.dma_start(xsT[:], xsT_d[:])

                # zero-init big stationaries (ones cols set below)
                nc.gpsimd.memset(hpa0[:], 0.0)
                nc.vector.memset(hpa0[:, :, :, 32:33], 1.0)
                nc.gpsimd.memset(hpa1[:], 0.0)
                nc.gpsimd.memset(paySrc[:], 0.0)

                # s0 rows for this core's 512 dst rows; a = e^{0.8 s}
                ps0 = pssa0.tile([8, 512], F32, tag="ps0")
                nc.tensor.matmul(ps0[:], w0s[:], xsT[:])
                nc.scalar.activation(a0row[:], ps0[:],
                                     mybir.ActivationFunctionType.Exp,
                                     scale=0.8)
                for h in range(8):
                    pa = pssa.tile([128, 512], F32, tag="pa")
                    nc.tensor.matmul(pa[:], sela[:, h * 128:(h + 1) * 128],
                                     a0row[:])
                    if h % 2 == 0:
                        nc.vector.tensor_copy(atile[:, h, :], pa[:])
                    else:
                        nc.scalar.copy(atile[:, h, :], pa[:])

                # h'0 (bf16) and raw d0 per j-tile
                for jt in range(NJT):
                    p80 = mm80.tile([128, 72], F32)
                    nc.tensor.matmul(p80[:], xT[:, jt * 128:(jt + 1) * 128],
                                     w0aug[:])
                    hsrc = p80[:, 0:64].rearrange("p (h o) -> p h o", h=8)
                    if jt % 2 == 0:
                        nc.vector.tensor_copy(hpa0[:, jt, :, 0:8], hsrc)
                        nc.scalar.copy(dstage[:, jt, :], p80[:, 64:72])
                    else:
                        nc.scalar.copy(hpa0[:, jt, :, 0:8], hsrc)
                        nc.vector.tensor_copy(dstage[:, jt, :], p80[:, 64:72])
                    if jt % 8 == 7:
                        b = jt - 7
                        nc.scalar.activation(
                            d0e[:, b:jt + 1, :], dstage[:, b:jt + 1, :],
                            mybir.ActivationFunctionType.Exp)
                        nc.scalar.activation(
                            d0e2[:, b:jt + 1, :], dstage[:, b:jt + 1, :],
                            mybir.ActivationFunctionType.Exp, scale=0.2)

            # ------- Phase B: layer-0 attention + local L1 projection -------
            with (
                tc.tile_pool(name="epool", bufs=14) as epool,
                tc.tile_pool(name="agg", bufs=3, space="PSUM") as agg,
                tc.tile_pool(name="prbp", bufs=2, space="PSUM") as prbp,
                tc.tile_pool(name="tmp", bufs=3) as tmp,
            ):
                pgs = {}

                def emit_head_tile(h, jt):
                    if jt == 0:
                        pgs[h] = agg.tile([33, 512], F32, name=f"pg{h}",
                                          tag="pg")
                    pg = pgs[h]
                    e = epool.tile([128, 512], BF16, tag="e")
                    nc.vector.tensor_scalar(
                        e[:], atile[:, h, :],
                        d0e[:, jt, h:h + 1], d0e2[:, jt, h:h + 1],
                        op0=mybir.AluOpType.mult,
                        op1=mybir.AluOpType.max)
                    nc.tensor.matmul(pg[:], hpa0[:, jt, h, 0:33], e[:],
                                     start=(jt == 0), stop=(jt == NJT - 1))

                def emit_norm(h, step):
                    """Normalize + bias + ELU for head h, split into 6 steps
                    so it interleaves with the next head's tile stream."""
                    ch, hh = h // 2, h % 2
                    pg = pgs[h]
                    st = norm_state[h]
                    if step == 0:
                        # stage den to SBUF (scalar engine), then fast
                        # approx reciprocal on DVE (~0.6us vs 3.3us exact;
                        # the custom DVE op needs an SBUF source)
                        st['den'] = tmp.tile([1, 512], F32, name="den",
                                             tag="den")
                        nc.scalar.copy(st['den'][:], pg[32:33, :])
                        st['rcp'] = tmp.tile([1, 512], F32, name="rcp",
                                             tag="rcp")
                        nc.vector.reciprocal_approx_fast(st['rcp'][:],
                                                         st['den'][:])
                    elif step == 1:
                        prb = prbp.tile([8, 512], F32)
                        nc.tensor.matmul(prb[:], ones8[:], st['rcp'][:])
                        st['prbs'] = tmp.tile([8, 512], F32, name="prbs", tag="prbs")
                        nc.scalar.copy(st['prbs'][:], prb[:])
                    elif step == 2:
                        st['nrm'] = tmp.tile([8, 512], F32, name="nrm", tag="nrm")
                        nc.vector.tensor_tensor(st['nrm'][:], pg[0:8, :],
                                                st['prbs'][:],
                                                op=mybir.AluOpType.mult)
                    elif step == 3:
                        # eneg = exp(min(nrm+b0, 0)) = exp(-relu(-nrm-b0))
                        st['mneg'] = tmp.tile([8, 512], F32, name="mneg", tag="mneg")
                        nc.scalar.activation(
                            st['mneg'][:], st['nrm'][:],
                            mybir.ActivationFunctionType.Relu,
                            bias=nb0p[:], scale=-1.0)
                        st['eneg'] = tmp.tile([8, 512], F32, name="eneg", tag="eneg")
                        nc.scalar.activation(
                            st['eneg'][:], st['mneg'][:],
                            mybir.ActivationFunctionType.Exp, scale=-1.0)
                    elif step == 4:
                        st['ppos'] = tmp.tile([8, 512], F32, name="ppos", tag="ppos")
                        nc.scalar.activation(
                            st['ppos'][:], st['nrm'][:],
                            mybir.ActivationFunctionType.Relu,
                            bias=b0p[:])
                    elif step == 5:
                        # elu half-row = (eneg - 1) + ppos
                        st['half'] = tmp.tile([8, 512], F32, name="half", tag="half")
                        nc.vector.scalar_tensor_tensor(
                            st['half'][:], st['eneg'][:], -1.0, st['ppos'][:],
                            op0=mybir.AluOpType.add,
                            op1=mybir.AluOpType.add)
                        nc.sync.dma_start(contc[hh * 8:(hh + 1) * 8, ch, :],
                                          st['half'][:])
                        if hh == 1:
                            # chunk complete: accumulate local L1 projection
                            # h'1^T += W1_ch^T @ contc_ch  and s1 partials
                            nc.tensor.matmul(p1T[:], w1ch[:, ch, 0:33],
                                             contc[:, ch, :],
                                             start=(ch == 0), stop=False)
                            nc.tensor.matmul(ps1[:], w1sc[:, ch:ch + 1],
                                             contc[:, ch, :],
                                             start=(ch == 0), stop=False)

                norm_state = [dict() for _ in range(8)]
                NORM_AT = [7, 11, 15, 19, 23, 27]  # jt positions in next head
                for h in range(8):
                    for jt in range(NJT):
                        emit_head_tile(h, jt)
                        if h > 0 and jt in NORM_AT:
                            emit_norm(h - 1, NORM_AT.index(jt))
                # head 7 norm: nothing left to interleave with
                for step in range(6):
                    emit_norm(7, step)

            # ---------------- Phase C: payload + gather ----------------
            with (
                tc.tile_pool(name="pd", bufs=2, space="PSUM") as pd,
                tc.tile_pool(name="tp2", bufs=2, space="PSUM") as tp2,
                tc.tile_pool(name="ot", bufs=2) as ot,
            ):
                # close the local L1 projection: bias row (beta1, db1) and
                # s1 bias, then exps
                nc.tensor.matmul(p1T[:], w1b[:, 0:33], ones512[:],
                                 start=False, stop=True)
                nc.tensor.matmul(ps1[:], sb1t[:], ones512[:],
                                 start=False, stop=True)
                nc.scalar.activation(a1row[:], ps1[:],
                                     mybir.ActivationFunctionType.Exp,
                                     scale=0.8)
                pa1 = pd.tile([128, 512], F32, tag="pa1")
                nc.tensor.matmul(pa1[:], ones_row_bf[:], a1row[:])
                nc.vector.tensor_copy(a1tile[:], pa1[:])

                # payload rows: 0:32 h'1 bf16, 32 ones, 33 e^{d1},
                # 34 e^{0.2 d1}, 35 pad
                nc.scalar.copy(paySrc[0:32, :], p1T[0:32, :])
                nc.vector.memset(paySrc[32:33, :], 1.0)
                # engine writes are limited to partition bases 0/32/64/96:
                # stage the exps at base 0 and DMA them into rows 33/34
                nc.scalar.activation(ed1st[:], p1T[32:33, :],
                                     mybir.ActivationFunctionType.Exp)
                nc.scalar.activation(ed2st[:], p1T[32:33, :],
                                     mybir.ActivationFunctionType.Exp,
                                     scale=0.2)
                nc.sync.dma_start(paySrc[33:34, :], ed1st[:])
                nc.sync.dma_start(paySrc[34:35, :], ed2st[:])

                if DEBUG:
                    nc.sync.dma_start(dbg_a0_d[:], a0row[:])
                    nc.sync.dma_start(dbg_ds_d[:], dstage[:, 0, :])
                    nc.sync.dma_start(dbg_hpa_d[:], hpa0[:, 0, 0, :])
                    nc.sync.dma_start(
                        dbg_cont_d[:],
                        contc[:].rearrange("p c f -> p (c f)"))
                    nc.sync.dma_start(dbg_pay_d[:], paySrc[:])
                for k in range(4):
                    pt = tp2.tile([128, 36], BF16)
                    nc.tensor.matmul(pt[:],
                                     paySrc[:, k * 128:(k + 1) * 128],
                                     identB[:], is_transpose=True)
                    nc.scalar.copy(payT[:, k, :], pt[:])
                nc.sync.dma_start(
                    contd1[:].rearrange("(k p) c -> p k c", p=128),
                    payT[:])
                nc.gpsimd.collective_compute(
                    "AllGather",
                    mybir.AluOpType.bypass,
                    replica_groups=[list(range(NCORES))],
                    ins=[contd1.opt()],
                    outs=[agc1.opt()],
                )
                nc.sync.dma_start(
                    hpa1[:],
                    agc1[:].rearrange("(k p) c -> p k c", p=128))
                nc.scalar.copy(d1e[:], hpa1[:, :, 33:34])
                nc.scalar.copy(d1e2[:], hpa1[:, :, 34:35])

            # ---------------- Phase D: layer-1 attention ----------------
            with (
                tc.tile_pool(name="e1pool", bufs=10) as e1pool,
                tc.tile_pool(name="agg1", bufs=1, space="PSUM") as agg1,
                tc.tile_pool(name="pd2", bufs=2, space="PSUM") as pd2,
                tc.tile_pool(name="tp3", bufs=2, space="PSUM") as tp3,
                tc.tile_pool(name="ot2", bufs=2) as ot2,
                tc.tile_pool(name="tmp2", bufs=2) as tmp2,
            ):
                pg1 = agg1.tile([33, 512], F32)
                for jt in range(NJT):
                    e1 = e1pool.tile([128, 512], BF16, tag="e1")
                    nc.vector.tensor_scalar(
                        e1[:], a1tile[:],
                        d1e[:, jt, :], d1e2[:, jt, :],
                        op0=mybir.AluOpType.mult,
                        op1=mybir.AluOpType.max)
                    nc.tensor.matmul(pg1[:], hpa1[:, jt, 0:33], e1[:],
                                     start=(jt == 0), stop=(jt == NJT - 1))

                if DEBUG:
                    nc.sync.dma_start(dbg_hpa1_d[:], hpa1[:, 0, :])
                    nc.sync.dma_start(dbg_a1_d[:], a1row[:])
                    pg1s = tmp2.tile([33, 512], F32, tag="pg1s")
                    nc.vector.tensor_copy(pg1s[:], pg1[:])
                    nc.sync.dma_start(dbg_pg1_d[:], pg1s[:])
                den1 = tmp2.tile([1, 512], F32, tag="den1")
                nc.scalar.copy(den1[:], pg1[32:33, :])
                rcp1 = tmp2.tile([1, 512], F32, tag="rcp1")
                nc.vector.reciprocal_approx_fast(rcp1[:], den1[:])
                prb1 = pd2.tile([32, 512], F32, tag="prb1")
                nc.tensor.matmul(prb1[:], ones32[:], rcp1[:])
                prbs1 = tmp2.tile([32, 512], F32, tag="prbs1")
                nc.scalar.copy(prbs1[:], prb1[:])
                nc.vector.tensor_tensor(o1s[:], pg1[0:32, :], prbs1[:],
                                        op=mybir.AluOpType.mult)
                nc.vector.tensor_scalar_add(o1s[:], o1s[:], b1c[:])

                for ic in range(4):
                    pt2 = tp3.tile([128, 32], F32)
                    nc.tensor.matmul(pt2[:],
                                     o1s[:, ic * 128:(ic + 1) * 128],
                                     ident[0:32, 0:32], is_transpose=True)
                    ob = ot2.tile([128, 32], F32, tag="ob")
                    nc.scalar.copy(ob[:], pt2[:])
                    nc.sync.dma_start(out_d[ic * 128:(ic + 1) * 128, :],
                                      ob[:])

    nc.compile()
    return nc


def _fold(inputs):
    """Host-side BN folding, attention-projection folding, x transpose."""
    f64 = np.float64
    x = np.asarray(inputs["x"], f64)
    w0 = np.asarray(inputs["w0"], f64)          # [8, 32, 8]
    w1 = np.asarray(inputs["w1"], f64)          # [1, 64, 32]
    a_src0 = np.asarray(inputs["a_src0"], f64)[..., 0]   # [8, 8]
    a_dst0 = np.asarray(inputs["a_dst0"], f64)[..., 0]   # [8, 8]
    a_src1 = np.asarray(inputs["a_src1"], f64)[0, :, 0]  # [32]
    a_dst1 = np.asarray(inputs["a_dst1"], f64)[0, :, 0]  # [32]
    b0 = np.asarray(inputs["b0"], f64)          # [8]
    b1 = np.asarray(inputs["b1"], f64)          # [32]

    al0 = np.asarray(inputs["bn0_gamma"], f64) / np.sqrt(
        np.asarray(inputs["bn0_var"], f64) + BN_EPS)
    sh0 = np.asarray(inputs["bn0_beta"], f64) - \
        np.asarray(inputs["bn0_mean"], f64) * al0
    al1 = np.asarray(inputs["bn1_gamma"], f64) / np.sqrt(
        np.asarray(inputs["bn1_var"], f64) + BN_EPS)
    sh1 = np.asarray(inputs["bn1_beta"], f64) - \
        np.asarray(inputs["bn1_mean"], f64) * al1

    # layer 0 folds
    w0flat = (al0[None, :, None] * w0).transpose(1, 0, 2).reshape(32, 64)
    beta0h = np.einsum("i,hio->ho", sh0, w0)     # [8, 8]
    beta0 = beta0h.reshape(64)
    as0 = al0[:, None] * np.einsum("hio,ho->ih", w0, a_src0)   # [32, 8]
    sb0 = np.einsum("ho,ho->h", beta0h, a_src0)
    ad0 = al0[:, None] * np.einsum("hio,ho->ih", w0, a_dst0)
    db0 = np.einsum("ho,ho->h", beta0h, a_dst0)

    w0aug = np.zeros((33, 72), f64)
    w0aug[0:32, 0:64] = w0flat
    w0aug[32, 0:64] = beta0
    w0aug[0:32, 64:72] = ad0
    w0aug[32, 64:72] = db0
    w0s = np.zeros((33, 8), f64)
    w0s[0:32, :] = as0
    w0s[32, :] = sb0

    # layer 1 folds
    w1m = w1[0]                                   # [64, 32]
    w1flat = al1[:, None] * w1m
    beta1 = sh1 @ w1m                             # [32]
    as1 = al1 * (w1m @ a_src1)
    sb1 = beta1 @ a_src1
    ad1 = al1 * (w1m @ a_dst1)
    db1 = beta1 @ a_dst1

    # per-chunk [16, 34] blocks: cols 0:32 w1, col 32 = ad1, col 33 pad
    w1ch = np.zeros((16, 4, 34), f64)
    for c in range(4):
        w1ch[:, c, 0:32] = w1flat[c * 16:(c + 1) * 16, :]
        w1ch[:, c, 32] = ad1[c * 16:(c + 1) * 16]
    w1bias = np.zeros((1, 34), f64)
    w1bias[0, 0:32] = beta1
    w1bias[0, 32] = db1

    w1sc = as1.reshape(4, 16).T                   # [16, 4]
    sb1t = np.array([[sb1]])

    sela = np.zeros((8, 8, 128), ml_dtypes.bfloat16)  # row h ones in block h
    for h in range(8):
        sela[h, h, :] = 1.0

    xT33 = np.zeros((33, N), np.float32)
    xT33[0:32, :] = x.T
    xT33[32, :] = 1.0

    return {
        "xT33_full": xT33,
        "w0aug": w0aug.astype(np.float32),
        "w0s": w0s.astype(np.float32),
        "w1ch": np.ascontiguousarray(w1ch.reshape(16, 4 * 34)
                                     ).astype(np.float32),
        "w1b": w1bias.astype(np.float32),
        "w1sc": w1sc.astype(np.float32),
        "sb1t": sb1t.astype(np.float32),
        "b0p": b0.reshape(8, 1).astype(np.float32),
        "nb0p": (-b0).reshape(8, 1).astype(np.float32),
        "b1f": b1.reshape(32, 1).astype(np.float32),
        "sela": sela.reshape(8, 8 * 128),
    }


def _in_maps(inputs):
    shared = _fold(inputs)
    xT33 = shared.pop("xT33_full")
    in_maps = []
    for c in range(NCORES):
        m = dict(shared)
        m["xT33"] = xT33
        m["xsT33"] = np.ascontiguousarray(xT33[:, c * RPC:(c + 1) * RPC])
        in_maps.append(m)
    return in_maps


def kernel(**inputs) -> np.ndarray:
    if "nc" not in _CACHE:
        _CACHE["nc"] = _build()
    nc = _CACHE["nc"]

    res = run_bass_kernel_spmd(nc, _in_maps(inputs), list(range(NCORES)))
    out = np.concatenate([res.results[c]["out"] for c in range(NCORES)],
                         axis=0)
    return out.astype(np.float32)
